# revision 3
# baseline (speedup 1.0000x reference)
"""BigBird attention (faithful .view-split variant) on 8 Trainium2 NeuronCores.

Sharding: the reference's `.reshape(B, H, S, hd)` head-split makes each
(batch, head) attend over a [2048, 64] row-major reshape of a 128-token
chunk's [128, 1024] projection. The 2*16 = 32 (b,h) pairs are sharded 4 per
core (batch x head parallel). The output projection is computed per-core as
a partial sum over its 4 heads (row-parallel over Wo), partials are summed
on the host.

Per core:
  A) QKV projections for its 4 chunks (fp32r matmuls), bounce to DRAM.
  B) Per chunk: block-sparse attention. Scores computed transposed
     (S^T strips, k on partitions) in fp32r; exp on ScalarE (scale=1/8
     folded in) to bf16 E strips; AV matmuls in bf16 with a ones column
     appended to V giving softmax sums for free; normalization via
     reciprocal + DMA partition-broadcast.
  C) Partial output projection y^T = sum_h Wo_h O_h^T with head pairs
     stacked on partitions (K=128, bf16).

The block mask (band + global cols 0/31 + 3 random blocks) is known at
trace time from src_blocks/tgt_blocks, so the sparsity plan is specialized
per call.
"""

import numpy as np
import ml_dtypes

import concourse.bass as bass
import concourse.mybir as mybir
import concourse.tile as tile
from concourse import bacc
from concourse.bass_utils import run_bass_kernel_spmd

B, S, DIM = 2, 2048, 1024
NHEADS, HD, BLK = 16, 64, 64
NB = S // BLK          # 32 block rows/cols
NCORES = 8
HPC = NHEADS * B // NCORES  # 4 chunks (b,h) per core
P = 128

f32 = mybir.dt.float32
f32r = mybir.dt.float32r
bf16 = mybir.dt.bfloat16

LAST_EXEC_NS = None
LAST_TRACE = None
LAST_INSTS = None


def _block_mask(src_blocks, tgt_blocks):
    i = np.arange(NB)[:, None]
    j = np.arange(NB)[None, :]
    bm = (np.abs(i - j) <= 1) | (j == 0) | (j == NB - 1)
    bm[np.asarray(src_blocks), np.asarray(tgt_blocks)] = True
    return bm


def _plan_strips(bm):
    """Cover the active blocks with k-stacked strips.

    Strip = dict(k=[kb...] (1 or 2 k-blocks stacked on partitions),
                 q0, qn (q-block run), act [len(k), qn] bool, kind).
    Active cells are claimed exactly once across strips so softmax sums
    are exact.
    """
    claimed = np.zeros((NB, NB), bool)
    strips = []
    # global columns 0 and 31, stacked, full q range
    strips.append(dict(k=[0, NB - 1], q0=0, qn=NB,
                       act=np.ones((2, NB), bool), kind="glob"))
    claimed[:, 0] = True
    claimed[:, NB - 1] = True
    # band strips: k-pair (2m-1, 2m), q-blocks [2m-2, 2m+2)
    for m in range(1, NB // 2):
        kbs = [2 * m - 1, 2 * m]
        q0, qn = 2 * m - 2, 4
        act = np.zeros((2, qn), bool)
        for ki, k in enumerate(kbs):
            for qi in range(qn):
                q = q0 + qi
                if bm[q, k] and not claimed[q, k]:
                    act[ki, qi] = True
                    claimed[q, k] = True
        strips.append(dict(k=kbs, q0=q0, qn=qn, act=act, kind="band"))
    # leftover random blocks
    rem = np.argwhere(bm & ~claimed)
    byk = {}
    for q, k in rem:
        byk.setdefault(int(k), []).append(int(q))
    for k, qs in sorted(byk.items()):
        qs = sorted(qs)
        while qs:
            q0 = min(max(qs[0] - 1, 0), NB - 4)
            qn = 4
            act = np.zeros((1, qn), bool)
            rest = []
            for q in qs:
                if q0 <= q < q0 + qn:
                    act[0, q - q0] = True
                    claimed[q, k] = True
                else:
                    rest.append(q)
            qs = rest
            strips.append(dict(k=[k], q0=q0, qn=qn, act=act, kind="extra"))
    return strips


def _build_program(strips, use_bias=True):
    nc = bacc.Bacc("TRN2", target_bir_lowering=False, debug=False,
                   num_devices=NCORES)

    # ---- per-core external inputs ----
    d_xt = nc.dram_tensor("xt", [HPC, P, DIM], f32r, kind="ExternalInput")
    d_wq = nc.dram_tensor("wq", [P, 8 * DIM], f32r, kind="ExternalInput")
    d_wk = nc.dram_tensor("wk", [P, 8 * DIM], f32r, kind="ExternalInput")
    d_wv = nc.dram_tensor("wv", [P, 8 * DIM], f32r, kind="ExternalInput")
    d_bq = nc.dram_tensor("bq", [1, DIM], f32, kind="ExternalInput")
    d_bk = nc.dram_tensor("bk", [1, DIM], f32, kind="ExternalInput")
    d_bv = nc.dram_tensor("bv", [1, DIM], f32, kind="ExternalInput")
    d_wo = nc.dram_tensor("wo", [2, P, DIM], bf16, kind="ExternalInput")
    d_yt = nc.dram_tensor("yt", [DIM, S], f32, kind="ExternalOutput")

    with tile.TileContext(nc) as tc:
        _emit(nc, tc, strips, d_xt, (d_wq, d_wk, d_wv),
              (d_bq, d_bk, d_bv), d_wo, d_yt, use_bias)
    nc.compile()
    return nc


def _emit(nc, tc, strips, d_xt, d_w, d_b, d_wo, d_yt, use_bias):
    from contextlib import ExitStack
    with ExitStack() as ctx:
        psA = ctx.enter_context(tc.tile_pool(name="psA", bufs=2, space="PSUM"))
        psS = ctx.enter_context(tc.tile_pool(name="psS", bufs=2, space="PSUM"))
        psOT = ctx.enter_context(tc.tile_pool(name="psOT", bufs=4, space="PSUM"))
        dram = ctx.enter_context(tc.tile_pool(name="dram", bufs=1, space="DRAM"))
        sbB = ctx.enter_context(tc.tile_pool(name="sbB", bufs=1))
        sbW = ctx.enter_context(tc.tile_pool(name="sbW", bufs=2))
        sbN = ctx.enter_context(tc.tile_pool(name="sbN", bufs=1))

        # DRAM scratch: per-chunk projection bounces
        dlin = {}
        for nm, shp in (("q", [S, P]), ("k", [S, P]), ("v", [P, DIM])):
            dlin[nm] = [dram.tile(shp, bf16, tag=f"d{nm}{i}",
                                  name=f"d{nm}{i}")
                        for i in range(HPC)]

        # ---------------- Phase A: QKV projections (proj-major) ----------
        with tc.tile_pool(name="wp", bufs=2) as wp, \
             tc.tile_pool(name="xp", bufs=1) as xp, \
             tc.tile_pool(name="bp", bufs=2) as bp, \
             tc.tile_pool(name="lp", bufs=2) as lp:
            xtiles = [xp.tile([P, DIM], f32r, tag=f"xt{i}", name=f"xt{i}")
                      for i in range(HPC)]
            for i in range(HPC):
                nc.sync.dma_start(xtiles[i][:], d_xt[i])
            for nm, dw, db in zip("qkv", d_w, d_b):
                w = wp.tile([P, 8 * DIM], f32r, tag="w")
                for kt in range(8):
                    nc.sync.dma_start(w[:, kt * DIM:(kt + 1) * DIM],
                                      dw[:, kt * DIM:(kt + 1) * DIM])
                bt = bp.tile([P, DIM], f32, tag="b")
                nc.sync.dma_start(bt[:], db[:].to_broadcast((P, DIM)))
                if nm == "v":
                    lint = lp.tile([P, DIM], bf16, tag="linv", name="lintv")
                else:
                    # d-axis padded to 128 (zeros) so the bounce is DMA-
                    # transposable: dram layout [s', 128] = [t, (c, d|pad)]
                    lint = lp.tile([P, 2 * DIM], bf16, tag=f"lin{nm}",
                                   name=f"lint{nm}")
                    nc.vector.memset(
                        lint[:].rearrange("p (c x) -> p c x",
                                          x=P)[:, :, 64:P], 0.0)
                for i in range(HPC):
                    xt = xtiles[i]
                    for nb2 in range(2):
                        ps = psA.tile([P, 512], f32, tag="mm512")
                        for kt in range(8):
                            nc.tensor.matmul(
                                ps[:],
                                lhsT=xt[:, kt * P:(kt + 1) * P],
                                rhs=w[:, kt * DIM + nb2 * 512:
                                      kt * DIM + nb2 * 512 + 512],
                                start=(kt == 0), stop=(kt == 7))
                        if nm == "v":
                            out_ap = lint[:, nb2 * 512:(nb2 + 1) * 512
                                          ].rearrange("p (c d) -> p c d", d=64)
                        else:
                            out_ap = lint[:].rearrange(
                                "p (c x) -> p c x",
                                x=P)[:, nb2 * 8:(nb2 + 1) * 8, 0:64]
                        if use_bias:
                            nc.vector.tensor_add(
                                out_ap,
                                ps[:].rearrange("p (c d) -> p c d", d=64),
                                bt[:, nb2 * 512:(nb2 + 1) * 512].rearrange(
                                    "p (c d) -> p c d", d=64))
                        else:
                            nc.scalar.copy(
                                out_ap,
                                ps[:].rearrange("p (c d) -> p c d", d=64))
                    nc.sync.dma_start(dlin[nm][i][:], lint[:])

        # Wo slices for phase C (loaded early, small)
        wob = sbB.tile([P, 2 * DIM], bf16, tag="wob")
        nc.sync.dma_start(wob[:, 0:DIM], d_wo[0])
        nc.sync.dma_start(wob[:, DIM:2 * DIM], d_wo[1])

        # O2 tiles: head-pair-stacked normalized O^T, consumed by phase C
        o2 = [sbB.tile([P, S], bf16, tag=f"o2_{a}", name=f"o2_{a}")
              for a in range(2)]

        # ---------------- Phase B: attention per chunk ----------------
        for i in range(HPC):
            qt = sbW.tile([P, S], bf16, tag="qt")
            nc.sync.dma_start(qt[:], dlin["q"][i][:], transpose=True)
            kt_ = sbW.tile([P, S], bf16, tag="kt")
            nc.sync.dma_start(kt_[:], dlin["k"][i][:], transpose=True)
            ktg = sbN.tile([P, P], bf16, tag="ktg")
            nc.sync.dma_start(ktg[:, 0:64], dlin["k"][i][0:64], transpose=True)
            nc.sync.dma_start(ktg[:, 64:128], dlin["k"][i][S - 64:S],
                              transpose=True)
            # V in band-pair layout: group g <-> k-blocks (2g+1, 2g+2)
            v2b = sbN.tile([P, 15 * 65], bf16, tag="v2b")
            nc.sync.dma_start(
                v2b[:].rearrange("p (g e) -> p g e", e=65)[:, :, 0:64],
                dlin["v"][i][4:124].rearrange("(g a) (b d) -> (a b) g d",
                                              a=8, d=64))
            nc.vector.memset(
                v2b[:].rearrange("p (g e) -> p g e", e=65)[:, :, 64:65], 1.0)
            # V glob pair: rows 0:64 = block 0, 64:128 = block 31, + ones col
            v2g = sbN.tile([P, 65], bf16, tag="v2g")
            nc.sync.dma_start(
                v2g[0:64, 0:64],
                dlin["v"][i][0:4].rearrange("t (c d) -> (t c) d", d=64))
            nc.sync.dma_start(
                v2g[64:128, 0:64],
                dlin["v"][i][124:128].rearrange("t (c d) -> (t c) d", d=64))
            nc.vector.memset(v2g[:, 64:65], 1.0)

            # --- strips: QK -> exp -> AV (interleaved) ---
            # AV accumulates O~^T (+ sums row 64) into psum [65, S].
            # Per psum bank: glob piece first (start=True); last piece per
            # bank gets stop=True (plan computed below).
            ot_h = [psOT.tile([65, 512], f32, tag="ot",
                                     name=f"ot{i}_{h}") for h in range(4)]
            npieces = [0] * 4   # total AV pieces per bank
            for st in strips:
                q = st["q0"] * BLK
                qhi = (st["q0"] + st["qn"]) * BLK
                while q < qhi:
                    bk2 = q // 512
                    qe = min(qhi, (bk2 + 1) * 512)
                    npieces[bk2] += 1
                    q = qe
            done = [0] * 4

            def av_pieces(st, lhs, et, pb, rows):
                qlo = st["q0"] * BLK
                qhi = (st["q0"] + st["qn"]) * BLK
                q = qlo
                while q < qhi:
                    bk2 = q // 512
                    qe = min(qhi, (bk2 + 1) * 512)
                    nc.tensor.matmul(
                        ot_h[bk2][0:65, q - bk2 * 512:qe - bk2 * 512],
                        lhsT=lhs,
                        rhs=et[pb:pb + rows, q - qlo:qe - qlo],
                        start=(done[bk2] == 0),
                        stop=(done[bk2] == npieces[bk2] - 1))
                    done[bk2] += 1
                    q = qe

            with tc.tile_pool(name=f"pe{i}", bufs=1) as pe:
                for si, st in enumerate(strips):
                    qlo, qn = st["q0"] * BLK, st["qn"] * BLK
                    if st["kind"] == "glob":
                        eg = pe.tile([P, S], bf16, tag="eg", name="eg")
                        for bk2 in range(4):
                            pss = psS.tile([P, 512], f32, tag="s")
                            nc.tensor.matmul(
                                pss[:], lhsT=ktg[0:64, :],
                                rhs=qt[0:64, bk2 * 512:(bk2 + 1) * 512],
                                start=True, stop=True)
                            nc.scalar.activation(
                                eg[:, bk2 * 512:(bk2 + 1) * 512], pss[:],
                                mybir.ActivationFunctionType.Exp, scale=0.125)
                        av_pieces(st, v2g[:], eg, 0, 128)
                    elif st["kind"] == "band":
                        k0 = st["k"][0] * BLK
                        em = pe.tile([P, 256], bf16, tag=f"es{si}",
                                     name=f"es{si}")
                        pss = psS.tile([P, 512], f32, tag="s")
                        nc.tensor.matmul(
                            pss[:, 0:qn], lhsT=kt_[0:64, k0:k0 + 128],
                            rhs=qt[0:64, qlo:qlo + qn],
                            start=True, stop=True)
                        nc.scalar.activation(
                            em[:, 0:qn], pss[:, 0:qn],
                            mybir.ActivationFunctionType.Exp, scale=0.125)
                        for ki in range(2):
                            for qi in range(st["qn"]):
                                if not st["act"][ki, qi]:
                                    nc.vector.memset(
                                        em[ki * 64:(ki + 1) * 64,
                                           qi * 64:(qi + 1) * 64], 0.0)
                        g = (st["k"][0] - 1) // 2
                        av_pieces(st, v2b[:, g * 65:(g + 1) * 65], em, 0, 128)
                    else:  # extra: single k-block, all at partition base 0
                        kb = st["k"][0]
                        vx = pe.tile([64, 65], bf16, tag=f"vx{si}",
                                     name=f"vx{si}")
                        nc.sync.dma_start(
                            vx[:, 0:64],
                            dlin["v"][i][kb * 4:kb * 4 + 4].rearrange(
                                "t (c d) -> (t c) d", d=64))
                        nc.vector.memset(vx[:, 64:65], 1.0)
                        ex = pe.tile([P, 256], bf16, tag=f"es{si}",
                                     name=f"es{si}")
                        nc.vector.memset(ex[0:64, 0:qn], 0.0)
                        pss = psS.tile([P, 512], f32, tag="s")
                        nc.tensor.matmul(
                            pss[0:64, 0:qn],
                            lhsT=kt_[0:64, kb * BLK:kb * BLK + 64],
                            rhs=qt[0:64, qlo:qlo + qn],
                            start=True, stop=True)
                        for qi in range(st["qn"]):
                            if st["act"][0, qi]:
                                nc.scalar.activation(
                                    ex[0:64, qi * 64:(qi + 1) * 64],
                                    pss[0:64, qi * 64:(qi + 1) * 64],
                                    mybir.ActivationFunctionType.Exp, scale=0.125)
                        av_pieces(st, vx[:], ex, 0, 64)

            # --- normalize -> O2 (per half, pipelined) ---
            # reshape the psum sums row to [128, 8] so reciprocal runs on
            # all lanes, then broadcast 1/s across 64 partitions via DRAM
            HF = 512
            a, half = i // 2, i % 2
            for h in range(4):
                oth = ot_h[h]
                srow = sbN.tile([65, HF], f32, tag=f"srow{h}",
                                name=f"srow{h}")
                nc.scalar.copy(srow[64:65, :], oth[64:65, :])
                dsum = dram.tile([1, HF], f32, tag=f"dsum{i % 2}{h}",
                                 name=f"dsum{i % 2}{h}")
                nc.sync.dma_start(dsum[:], srow[64:65, :])
                ssum = sbN.tile([P, 4], f32, tag=f"ssum{h}", name=f"ssum{h}")
                nc.sync.dma_start(
                    ssum[:], dsum[:].rearrange("o (p f) -> (o p) f", f=4))
                rr = sbN.tile([P, 4], f32, tag=f"rr{h}", name=f"rr{h}")
                nc.vector.reciprocal(rr[:], ssum[:])
                drr = dram.tile([1, HF], f32, tag=f"drr{i % 2}{h}",
                                name=f"drr{i % 2}{h}")
                nc.sync.dma_start(
                    drr[:].rearrange("o (p f) -> (o p) f", f=4), rr[:])
                rbc = sbN.tile([64, HF], f32, tag=f"rbc{h}", name=f"rbc{h}")
                nc.sync.dma_start(rbc[:], drr[:].to_broadcast((64, HF)))
                if half == 0:
                    nc.vector.tensor_mul(o2[a][0:64, h * HF:(h + 1) * HF],
                                         oth[0:64, :], rbc[:])
                else:
                    o2t = sbN.tile([64, HF], bf16, tag=f"o2t{h}",
                                   name=f"o2t{h}")
                    nc.vector.tensor_mul(o2t[:], oth[0:64, :], rbc[:])
                    nc.sync.dma_start(o2[a][64:128, h * HF:(h + 1) * HF],
                                      o2t[:])

        # ---------------- Phase C: partial output projection ----------------
        with tc.tile_pool(name="yp", bufs=3) as yp:
            for qb in range(4):
                for mt in range(8):
                    ps = psA.tile([P, 512], f32, tag="mm512")
                    for a in range(2):
                        nc.tensor.matmul(
                            ps[:],
                            lhsT=wob[:, a * DIM + mt * P: a * DIM + (mt + 1) * P],
                            rhs=o2[a][:, qb * 512:(qb + 1) * 512],
                            start=(a == 0), stop=(a == 1))
                    yt = yp.tile([P, 512], f32, tag="yt")
                    nc.scalar.copy(yt[:], ps[:])
                    nc.sync.dma_start(
                        d_yt[mt * P:(mt + 1) * P, qb * 512:(qb + 1) * 512],
                        yt[:])


def kernel(x, Wq, bq, Wk, bk, Wv, bv, Wo, bo, src_blocks, tgt_blocks,
           _trace=False):
    global LAST_EXEC_NS, LAST_TRACE
    x = np.asarray(x, np.float32)
    bm = _block_mask(np.asarray(src_blocks), np.asarray(tgt_blocks))
    strips = _plan_strips(bm)
    use_bias = bool(np.any(np.asarray(bq)) or np.any(np.asarray(bk))
                    or np.any(np.asarray(bv)))
    nc = _build_program(strips, use_bias)

    # host-side shard prep
    # W layout for rhs: w[p, kt*1024 + j] = W[j, kt*128 + p]
    def w_rhs(W):
        Wt = np.ascontiguousarray(np.asarray(W, np.float32).T)  # [in, out]
        return np.ascontiguousarray(
            Wt.reshape(8, P, DIM).transpose(1, 0, 2).reshape(P, 8 * DIM))

    wq_h, wk_h, wv_h = w_rhs(Wq), w_rhs(Wk), w_rhs(Wv)
    WoT = np.asarray(Wo, np.float32).T  # [in(=64*head), out]
    x4 = x.reshape(B, NHEADS, P, DIM)

    in_maps = []
    for c in range(NCORES):
        b = c // 4
        h0 = 4 * (c % 4)
        xc = x4[b, h0:h0 + 4]                       # [4, 128, 1024]
        xt = np.ascontiguousarray(xc.transpose(0, 2, 1))  # [4, 1024, 128]
        # xt dram layout [4, 128, 8*128]: xts[i, p, kt*128+t] = x[t, kt*128+p]
        xts = np.ascontiguousarray(
            xt.reshape(HPC, 8, P, P).transpose(0, 2, 1, 3).reshape(HPC, P, 8 * P))
        wo_c = np.zeros((2, P, DIM), ml_dtypes.bfloat16)
        for a in range(2):
            r0 = 64 * (h0 + 2 * a)
            wo_c[a] = WoT[r0:r0 + 128].astype(ml_dtypes.bfloat16)
        in_maps.append({
            "xt": xts,
            "wq": wq_h, "wk": wk_h, "wv": wv_h,
            "bq": np.asarray(bq, np.float32).reshape(1, DIM),
            "bk": np.asarray(bk, np.float32).reshape(1, DIM),
            "bv": np.asarray(bv, np.float32).reshape(1, DIM),
            "wo": wo_c,
        })

    if _trace:
        try:
            import sys
            sys.path.insert(0, "/root/problem/work")
            import ntff_shim
            ntff_shim.install()
        except Exception:
            pass
    res = run_bass_kernel_spmd(nc, in_maps, core_ids=list(range(NCORES)),
                               trace=_trace)
    LAST_EXEC_NS = res.exec_time_ns
    LAST_TRACE = (res.instructions_and_trace[1]
                  if res.instructions_and_trace else None)
    global LAST_INSTS
    LAST_INSTS = (res.instructions_and_trace[0]
                  if res.instructions_and_trace else None)

    y = np.zeros((B, S, DIM), np.float32)
    for c in range(NCORES):
        y[c // 4] += res.results[c]["yt"].T
    y += np.asarray(bo, np.float32)
    return y



# revision 18
# speedup vs baseline: 1.1246x; 1.1246x over previous
"""BigBird attention (faithful .view-split variant) on 8 Trainium2 NeuronCores.

Sharding: the reference's `.reshape(B, H, S, hd)` head-split makes each
(batch, head) attend over a [2048, 64] row-major reshape of a 128-token
chunk's [128, 1024] projection. The 2*16 = 32 (b,h) pairs are sharded 4 per
core (batch x head parallel). The output projection is computed per-core as
a partial sum over its 4 heads (row-parallel over Wo), partials are summed
on the host.

Everything runs in bf16 on the PE (fp32 would double-pump the array).
Chunks are processed in PAIRS (A, B): their q^T/k^T share one [128, 2048]
SBUF tile (A on partitions 0:64, B on 64:128) produced by a single packed
DRAM bounce + DMA-transpose, and the block-sparse QK^T matmuls for A and B
run concurrently on the PE via row-tiling (tile_position row groups 0/64).
Scores go through one exp per strip covering both chunks; AV accumulates
O~^T (+softmax sums via a ones column on V) per 512-query psum bank.
Normalization is all on-chip: 1-lane reciprocal of the sums row, then a
K=1 PE matmul broadcasts 1/s across 64 partitions; V strips are filled by
SBUF->SBUF DMA from the projection output (no DRAM round trip).

The block mask (band + global cols 0/31 + 3 random blocks) is known at
trace time from src_blocks/tgt_blocks, so the sparsity plan is specialized
per call.
"""

import numpy as np
import ml_dtypes

import concourse.bass as bass
import concourse.mybir as mybir
import concourse.tile as tile
from concourse import bacc
from concourse.bass_utils import run_bass_kernel_spmd

B, S, DIM = 2, 2048, 1024
NHEADS, HD, BLK = 16, 64, 64
NB = S // BLK          # 32 block rows/cols
NCORES = 8
HPC = NHEADS * B // NCORES  # 4 chunks (b,h) per core
P = 128
QT = 512               # query columns per psum bank

f32 = mybir.dt.float32
bf16 = mybir.dt.bfloat16

LAST_EXEC_NS = None
LAST_TRACE = None
LAST_INSTS = None


def _block_mask(src_blocks, tgt_blocks):
    i = np.arange(NB)[:, None]
    j = np.arange(NB)[None, :]
    bm = (np.abs(i - j) <= 1) | (j == 0) | (j == NB - 1)
    bm[np.asarray(src_blocks), np.asarray(tgt_blocks)] = True
    return bm


def _plan_strips(bm):
    """Cover the active blocks with k-stacked strips.

    Strip = dict(k=[kb...] (1 or 2 k-blocks stacked on partitions),
                 q0, qn (q-block run), act [len(k), qn] bool, kind).
    Active cells are claimed exactly once across strips so softmax sums
    are exact.  Global columns 0/31 are handled separately (per q-tile).
    """
    claimed = np.zeros((NB, NB), bool)
    strips = []
    claimed[:, 0] = True
    claimed[:, NB - 1] = True
    # band strips: k-pair (2m-1, 2m), q-blocks [2m-2, 2m+2)
    for m in range(1, NB // 2):
        kbs = [2 * m - 1, 2 * m]
        q0, qn = 2 * m - 2, 4
        act = np.zeros((2, qn), bool)
        for ki, k in enumerate(kbs):
            for qi in range(qn):
                q = q0 + qi
                if bm[q, k] and not claimed[q, k]:
                    act[ki, qi] = True
                    claimed[q, k] = True
        strips.append(dict(k=kbs, q0=q0, qn=qn, act=act, kind="band"))
    # leftover random blocks
    rem = np.argwhere(bm & ~claimed)
    byk = {}
    for q, k in rem:
        byk.setdefault(int(k), []).append(int(q))
    for k, qs in sorted(byk.items()):
        qs = sorted(qs)
        while qs:
            q0 = min(max(qs[0] - 1, 0), NB - 4)
            qn = 4
            act = np.zeros((1, qn), bool)
            rest = []
            for q in qs:
                if q0 <= q < q0 + qn:
                    act[0, q - q0] = True
                    claimed[q, k] = True
                else:
                    rest.append(q)
            qs = rest
            strips.append(dict(k=[k], q0=q0, qn=qn, act=act, kind="extra"))
    strips.sort(key=lambda st: st["q0"])
    return strips


def _build_program(strips, use_bias=True):
    nc = bacc.Bacc("TRN2", target_bir_lowering=False, debug=False,
                   num_devices=NCORES)

    d_xt = nc.dram_tensor("xt", [HPC, P, 8 * P], bf16, kind="ExternalInput")
    d_wq = nc.dram_tensor("wq", [P, 8 * DIM], bf16, kind="ExternalInput")
    d_wk = nc.dram_tensor("wk", [P, 8 * DIM], bf16, kind="ExternalInput")
    d_wv = nc.dram_tensor("wv", [P, 8 * DIM], bf16, kind="ExternalInput")
    d_bq = nc.dram_tensor("bq", [1, DIM], f32, kind="ExternalInput")
    d_bk = nc.dram_tensor("bk", [1, DIM], f32, kind="ExternalInput")
    d_bv = nc.dram_tensor("bv", [1, DIM], f32, kind="ExternalInput")
    d_wo = nc.dram_tensor("wo", [2, P, DIM], bf16, kind="ExternalInput")
    d_yt = nc.dram_tensor("yt", [DIM, S], bf16, kind="ExternalOutput")

    with tile.TileContext(nc) as tc:
        _emit(nc, tc, strips, d_xt, (d_wq, d_wk, d_wv),
              (d_bq, d_bk, d_bv), d_wo, d_yt, use_bias)
    nc.compile()
    return nc


def _bank_pieces(strips):
    """Total AV pieces per 512-query psum bank (glob contributes 1 each)."""
    npieces = [1] * 4
    for st in strips:
        q = st["q0"] * BLK
        qhi = (st["q0"] + st["qn"]) * BLK
        while q < qhi:
            u = q // QT
            qe = min(qhi, (u + 1) * QT)
            npieces[u] += 1
            q = qe
    return npieces


def _emit(nc, tc, strips, d_xt, d_w, d_b, d_wo, d_yt, use_bias):
    from contextlib import ExitStack
    Exp = mybir.ActivationFunctionType.Exp
    with ExitStack() as ctx:
        # PSUM: 2 (proj/out) + 3 (O^T accum) + 3 (scores/broadcast) = 8 banks
        psA = ctx.enter_context(tc.tile_pool(name="psA", bufs=2, space="PSUM"))
        psOT = ctx.enter_context(tc.tile_pool(name="psOT", bufs=4,
                                              space="PSUM"))
        psS = ctx.enter_context(tc.tile_pool(name="psS", bufs=1, space="PSUM"))
        dram = ctx.enter_context(tc.tile_pool(name="dram", bufs=1,
                                              space="DRAM"))
        sbB = ctx.enter_context(tc.tile_pool(name="sbB", bufs=1))
        sbW = ctx.enter_context(tc.tile_pool(name="sbW", bufs=2))
        sbE = ctx.enter_context(tc.tile_pool(name="sbE", bufs=3))
        sbN = ctx.enter_context(tc.tile_pool(name="sbN", bufs=2))

        # weights + constants
        wts = {}
        for nm, dw in zip("qkv", d_w):
            w = sbB.tile([P, 8 * DIM], bf16, tag=f"w{nm}", name=f"w{nm}")
            for kt in range(8):
                nc.sync.dma_start(w[:, kt * DIM:(kt + 1) * DIM],
                                  dw[:, kt * DIM:(kt + 1) * DIM])
            wts[nm] = w
        wob = sbB.tile([P, 2 * DIM], bf16, tag="wob")
        nc.sync.dma_start(wob[:, 0:DIM], d_wo[0])
        nc.sync.dma_start(wob[:, DIM:2 * DIM], d_wo[1])
        ones = sbB.tile([P, BLK], bf16, tag="ones")
        nc.vector.memset(ones[:], 1.0)
        bts = {}
        if use_bias:
            for nm, db in zip("qkv", d_b):
                bt = sbB.tile([P, DIM], f32, tag=f"b{nm}", name=f"b{nm}")
                nc.sync.dma_start(bt[:], db[:].to_broadcast((P, DIM)))
                bts[nm] = bt
        xtiles = [sbB.tile([P, 8 * P], bf16, tag=f"xt{i}", name=f"xt{i}")
                  for i in range(HPC)]
        for i in range(HPC):
            nc.sync.dma_start(xtiles[i][:], d_xt[i])

        # o2[a]: head-pair-stacked normalized O^T for phase C
        o2 = [sbB.tile([P, S], bf16, tag=f"o2_{a}", name=f"o2_{a}")
              for a in range(2)]

        npieces = _bank_pieces(strips)

        for pr in range(2):          # chunk pair (2*pr, 2*pr+1)
            # ---------------- Phase A: QKV projections -------------------
            lint = {"q": sbW.tile([P, 2 * DIM], bf16, tag="lq", name=f"lq{pr}"),
                    "k": sbW.tile([P, 2 * DIM], bf16, tag="lk", name=f"lk{pr}")}
            lv = [sbW.tile([P, DIM], bf16, tag=f"lv{h}", name=f"lv{h}{pr}")
                  for h in range(2)]
            for nm in "qkv":
                w = wts[nm]
                for half in range(2):
                    i = 2 * pr + half
                    xt = xtiles[i]
                    for nb2 in range(2):
                        ps = psA.tile([P, 512], f32, tag="mm512")
                        for kt in range(8):
                            nc.tensor.matmul(
                                ps[:],
                                lhsT=xt[:, kt * P:(kt + 1) * P],
                                rhs=w[:, kt * DIM + nb2 * 512:
                                      kt * DIM + nb2 * 512 + 512],
                                start=(kt == 0), stop=(kt == 7))
                        if nm == "v":
                            out_ap = lv[half][:, nb2 * 512:(nb2 + 1) * 512
                                              ].rearrange("p (c d) -> p c d",
                                                          d=64)
                        else:
                            out_ap = lint[nm][:].rearrange(
                                "p (c x) -> p c x",
                                x=P)[:, nb2 * 8:(nb2 + 1) * 8,
                                     half * 64:(half + 1) * 64]
                        src = ps[:].rearrange("p (c d) -> p c d", d=64)
                        if use_bias:
                            nc.vector.tensor_add(
                                out_ap, src,
                                bts[nm][:, nb2 * 512:(nb2 + 1) * 512
                                        ].rearrange("p (c d) -> p c d", d=64))
                        else:
                            nc.vector.tensor_copy(out_ap, src)
            # bounce q,k through DRAM to transpose; v stays in SBUF
            qkt = {}
            for nm in "qk":
                dl = dram.tile([S, P], bf16, tag=f"d{nm}{pr}",
                               name=f"d{nm}{pr}")
                nc.sync.dma_start(dl[:], lint[nm][:])
                t = sbW.tile([P, S], bf16, tag=f"{nm}t", name=f"{nm}t{pr}")
                nc.sync.dma_start(t[:], dl[:], transpose=True)
                qkt[nm] = t
            qt_, kt_ = qkt["q"], qkt["k"]

            # V strips: bounce through DRAM (partition-scatter APs are only
            # legal with a DRAM side), global/extra tiles via flat-order
            # SBUF->SBUF (partition-major element zip, like the lint store)
            v2b, v2g, dvs = [], [], []
            for half in range(2):
                dv = dram.tile([P, DIM], bf16, tag=f"dv{half}{pr}",
                               name=f"dv{half}{pr}")
                nc.sync.dma_start(dv[:], lv[half][:])
                vb = sbN.tile([P, 15 * 65], bf16, tag=f"v2b{half}",
                              name=f"v2b{half}{pr}")
                nc.sync.dma_start(
                    vb[:].rearrange("p (g e) -> p g e", e=65)[:, :, 0:64],
                    dv[4:124].rearrange("(g a) (b d) -> (a b) g d",
                                        a=8, d=64))
                nc.vector.memset(
                    vb[:].rearrange("p (g e) -> p g e", e=65)[:, :, 64:65],
                    1.0)
                v2b.append(vb)
                vg = sbN.tile([P, 65], bf16, tag=f"v2g{half}",
                              name=f"v2g{half}{pr}")
                nc.sync.dma_start(
                    vg[0:64, 0:64],
                    dv[0:4].rearrange("t (c d) -> (t c) d", d=64))
                nc.sync.dma_start(
                    vg[64:128, 0:64],
                    dv[124:128].rearrange("t (c d) -> (t c) d", d=64))
                nc.vector.memset(vg[:, 64:65], 1.0)
                v2g.append(vg)
                dvs.append(dv)
            # extra-strip V tiles: A on rows 0:64, B on rows 64:128
            vxs = {}
            for si, st in enumerate(strips):
                if st["kind"] != "extra":
                    continue
                kb = st["k"][0]
                vx = sbN.tile([P, 65], bf16, tag=f"vx{si}", name=f"vx{si}{pr}")
                for half in range(2):
                    nc.sync.dma_start(
                        vx[half * 64:(half + 1) * 64, 0:64],
                        dvs[half][kb * 4:kb * 4 + 4].rearrange(
                            "t (c d) -> (t c) d", d=64))
                nc.vector.memset(vx[:, 64:65], 1.0)
                vxs[si] = vx

            # ---------------- Phase B: attention, q-tile major -----------
            # Concurrent row-tiled matmuls (different tile_position rows)
            # must NOT write the same PSUM bank at the same partitions —
            # that faults on HW.  Chunk A scores go to bank 0 of a 2-bank
            # [128, 1024] tile, chunk B to bank 1; one exp covers both.
            ots = {}          # (half, u) -> psum tile [65, 512]
            done = {}
            egs = {}          # u -> eg sbuf tile (glob E, both halves)

            def get_ot(half, u):
                # Banks must be OPENED by a full-width start piece so the
                # accumulate-vs-overwrite state stays uniform per bank: the
                # global-columns AV piece (all 512 q) plays that role.
                if (half, u) not in ots:
                    if u not in egs:
                        emit_glob_scores(u)
                    eg = egs[u]
                    ot = psOT.tile([65, QT], f32, tag="ot",
                                   name=f"ot{pr}{half}{u}")
                    ots[(half, u)] = ot
                    done[(half, u)] = 0
                    nc.tensor.matmul(
                        ot[0:65, :], lhsT=v2g[half][:],
                        rhs=eg[:, half * QT:(half + 1) * QT],
                        start=True, stop=(npieces[u] == 1))
                    done[(half, u)] = 1
                    if done[(half, u)] == npieces[u]:
                        finish(half, u)
                return ots[(half, u)]

            def emit_glob_scores(u):
                pg = psS.tile([P, 2 * QT], f32, tag="s", name=f"g{u}")
                eg = sbE.tile([P, 2 * QT], bf16, tag="eg", name=f"eg{u}")
                for half in range(2):
                    h64 = half * 64
                    off = half * QT
                    qcols = qt_[h64:h64 + 64, u * QT:(u + 1) * QT]
                    nc.tensor.matmul(pg[0:64, off:off + QT],
                                     lhsT=kt_[h64:h64 + 64, 0:64],
                                     rhs=qcols, start=True, stop=True)
                    nc.tensor.matmul(pg[64:128, off:off + QT],
                                     lhsT=kt_[h64:h64 + 64, S - 64:S],
                                     rhs=qcols, start=True, stop=True)
                nc.scalar.activation(eg[:], pg[:], Exp, scale=0.125)
                egs[u] = eg

            def finish(half, u):
                # normalize O~^T by the sums row, write into o2[pr]
                import os
                ot = ots.pop((half, u))
                if os.environ.get("BB_NORM_DRAM"):
                    # baseline-style: bounce sums through DRAM to reshape and
                    # broadcast across partitions
                    srow = sbN.tile([65, QT], f32, tag="srow",
                                    name=f"sr{half}{u}")
                    nc.scalar.copy(srow[64:65, :], ot[64:65, :])
                    dsum = dram.tile([1, QT], f32, tag=f"ds{half}{u}",
                                     name=f"ds{half}{u}")
                    nc.sync.dma_start(dsum[:], srow[64:65, :])
                    ssum = sbN.tile([P, 4], f32, tag="ssum",
                                    name=f"ss{half}{u}")
                    nc.sync.dma_start(
                        ssum[:], dsum[:].rearrange("o (p f) -> (o p) f", f=4))
                    rr4 = sbN.tile([P, 4], f32, tag="rr4", name=f"r4{half}{u}")
                    nc.vector.reciprocal(rr4[:], ssum[:])
                    drr = dram.tile([1, QT], f32, tag=f"dr{half}{u}",
                                    name=f"dr{half}{u}")
                    nc.sync.dma_start(
                        drr[:].rearrange("o (p f) -> (o p) f", f=4), rr4[:])
                    rbc = sbN.tile([64, QT], f32, tag="rbc",
                                   name=f"rbc{half}{u}")
                    nc.sync.dma_start(rbc[:], drr[:].to_broadcast((64, QT)))
                else:
                    rr = sbN.tile([65, QT], bf16, tag="rr",
                                  name=f"rr{half}{u}")
                    with nc.allow_low_precision(reason="1/s bcast in bf16"):
                        nc.vector.reciprocal(rr[64:65, :], ot[64:65, :])
                    rbp = psA.tile([P, QT], f32, tag="mm512",
                                   name=f"rbp{half}{u}")
                    nc.tensor.matmul(rbp[0:64, :], lhsT=ones[64:65, 0:64],
                                     rhs=rr[64:65, :], start=True, stop=True)
                    rbc = sbN.tile([64, QT], f32, tag="rbc",
                                   name=f"rbc{half}{u}")
                    nc.vector.tensor_copy(rbc[:], rbp[0:64, :])
                cols = slice(u * QT, (u + 1) * QT)
                if half == 0:
                    nc.vector.tensor_mul(o2[pr][0:64, cols],
                                         ot[0:64, :], rbc[:])
                else:
                    o2t = sbN.tile([64, QT], bf16, tag="o2t",
                                   name=f"o2t{half}{u}")
                    nc.vector.tensor_mul(o2t[:], ot[0:64, :], rbc[:])
                    nc.sync.dma_start(o2[pr][64:128, cols], o2t[:])

            def av_pieces(st_q0, st_qn, lhs, et, ecol, erow, rows):
                # accumulate AV pieces into the per-u psum banks (both halves
                # share the strip's E tile; `erow`/`ecol` locate each half's
                # scores inside it)
                for half in range(2):
                    qlo = st_q0 * BLK
                    qhi = (st_q0 + st_qn) * BLK
                    q = qlo
                    while q < qhi:
                        u = q // QT
                        qe = min(qhi, (u + 1) * QT)
                        ot = get_ot(half, u)
                        nc.tensor.matmul(
                            ot[0:65, q - u * QT:qe - u * QT],
                            lhsT=lhs[half],
                            rhs=et[erow[half]:erow[half] + rows,
                                   ecol[half] + q - qlo:ecol[half] + qe - qlo],
                            start=(done[(half, u)] == 0),
                            stop=(done[(half, u)] == npieces[u] - 1))
                        done[(half, u)] += 1
                        if done[(half, u)] == npieces[u]:
                            finish(half, u)
                        q = qe

            strips_by_u = [[] for _ in range(4)]
            for si, st in enumerate(strips):
                strips_by_u[st["q0"] * BLK // QT].append((si, st))

            for u in range(4):
                # open this q-tile's banks (emits the glob pieces)
                for half in range(2):
                    get_ot(half, u)

                # --- band strips, paired two per 2-bank scores tile ---
                bands = [s for s in strips_by_u[u] if s[1]["kind"] == "band"]
                extras = [s for s in strips_by_u[u] if s[1]["kind"] != "band"]
                for gi in range(0, len(bands), 2):
                    grp = bands[gi:gi + 2]
                    pss = psS.tile([P, 2 * QT], f32, tag="s",
                                   name=f"s{grp[0][0]}")
                    em = sbE.tile([P, 2 * QT], bf16, tag="em",
                                  name=f"em{grp[0][0]}")
                    for sl, (si, st) in enumerate(grp):
                        qlo, qn = st["q0"] * BLK, st["qn"] * BLK
                        k0 = st["k"][0] * BLK
                        for half in range(2):
                            h64 = half * 64
                            c0 = half * QT + sl * 256
                            nc.tensor.matmul(
                                pss[:, c0:c0 + qn],
                                lhsT=kt_[h64:h64 + 64, k0:k0 + 128],
                                rhs=qt_[h64:h64 + 64, qlo:qlo + qn],
                                start=True, stop=True)
                    wcols = 256 * len(grp)
                    nc.scalar.activation(em[:, 0:wcols], pss[:, 0:wcols],
                                         Exp, scale=0.125)
                    nc.scalar.activation(em[:, QT:QT + wcols],
                                         pss[:, QT:QT + wcols],
                                         Exp, scale=0.125)
                    for sl, (si, st) in enumerate(grp):
                        for half in range(2):
                            for ki in range(2):
                                for qi in range(st["qn"]):
                                    if not st["act"][ki, qi]:
                                        c0 = half * QT + sl * 256 + qi * 64
                                        nc.vector.memset(
                                            em[ki * 64:(ki + 1) * 64,
                                               c0:c0 + 64], 0.0)
                        g = (st["k"][0] - 1) // 2
                        lhs = [v2b[h][:, g * 65:(g + 1) * 65]
                               for h in range(2)]
                        av_pieces(st["q0"], st["qn"], lhs, em,
                                  ecol=(sl * 256, QT + sl * 256),
                                  erow=(0, 0), rows=128)

                # --- extra strips (single k-block), A and B quadrants of
                # one bank at disjoint partition ranges ---
                for si, st in extras:
                    qlo, qn = st["q0"] * BLK, st["qn"] * BLK
                    kb = st["k"][0]
                    psx = psS.tile([P, 2 * QT], f32, tag="s", name=f"s{si}")
                    ex = sbE.tile([P, 512], bf16, tag="ex", name=f"ex{si}")
                    for half in range(2):
                        h64 = half * 64
                        nc.vector.memset(
                            ex[h64:h64 + 64,
                               half * 256:half * 256 + qn], 0.0)
                        nc.tensor.matmul(
                            psx[h64:h64 + 64,
                                half * 256:half * 256 + qn],
                            lhsT=kt_[h64:h64 + 64,
                                     kb * BLK:kb * BLK + 64],
                            rhs=qt_[h64:h64 + 64, qlo:qlo + qn],
                            start=True, stop=True)
                        for qi in range(st["qn"]):
                            if st["act"][0, qi]:
                                nc.scalar.activation(
                                    ex[h64:h64 + 64,
                                       half * 256 + qi * 64:
                                       half * 256 + (qi + 1) * 64],
                                    psx[h64:h64 + 64,
                                        half * 256 + qi * 64:
                                        half * 256 + (qi + 1) * 64],
                                    Exp, scale=0.125)
                    lhs = [vxs[si][0:64, :], vxs[si][64:128, :]]
                    av_pieces(st["q0"], st["qn"], lhs, ex,
                              ecol=(0, 256), erow=(0, 64), rows=64)

        # ---------------- Phase C: partial output projection -------------
        with tc.tile_pool(name="yp", bufs=3) as yp:
            for qb in range(4):
                for mt in range(8):
                    ps = psA.tile([P, 512], f32, tag="mm512")
                    for a in range(2):
                        nc.tensor.matmul(
                            ps[:],
                            lhsT=wob[:, a * DIM + mt * P:
                                     a * DIM + (mt + 1) * P],
                            rhs=o2[a][:, qb * 512:(qb + 1) * 512],
                            start=(a == 0), stop=(a == 1))
                    yt = yp.tile([P, 512], bf16, tag="yt")
                    nc.vector.tensor_copy(yt[:], ps[:])
                    nc.sync.dma_start(
                        d_yt[mt * P:(mt + 1) * P, qb * 512:(qb + 1) * 512],
                        yt[:])


def kernel(x, Wq, bq, Wk, bk, Wv, bv, Wo, bo, src_blocks, tgt_blocks,
           _trace=False):
    global LAST_EXEC_NS, LAST_TRACE, LAST_INSTS
    x = np.asarray(x, np.float32)
    bm = _block_mask(np.asarray(src_blocks), np.asarray(tgt_blocks))
    strips = _plan_strips(bm)
    use_bias = bool(np.any(np.asarray(bq)) or np.any(np.asarray(bk))
                    or np.any(np.asarray(bv)))
    nc = _build_program(strips, use_bias)

    # host-side shard prep
    # W layout for rhs: w[p, kt*1024 + j] = W[j, kt*128 + p]
    def w_rhs(W):
        Wt = np.ascontiguousarray(np.asarray(W, np.float32).T)  # [in, out]
        return np.ascontiguousarray(
            Wt.reshape(8, P, DIM).transpose(1, 0, 2).reshape(P, 8 * DIM)
        ).astype(ml_dtypes.bfloat16)

    wq_h, wk_h, wv_h = w_rhs(Wq), w_rhs(Wk), w_rhs(Wv)
    WoT = np.asarray(Wo, np.float32).T  # [in(=64*head), out]
    x4 = x.reshape(B, NHEADS, P, DIM)

    in_maps = []
    for c in range(NCORES):
        b = c // 4
        h0 = 4 * (c % 4)
        xc = x4[b, h0:h0 + 4]                       # [4, 128, 1024]
        xt = np.ascontiguousarray(xc.transpose(0, 2, 1))  # [4, 1024, 128]
        # xt dram layout [4, 128, 8*128]: xts[i, p, kt*128+t] = x[t, kt*128+p]
        xts = np.ascontiguousarray(
            xt.reshape(HPC, 8, P, P).transpose(0, 2, 1, 3).reshape(
                HPC, P, 8 * P)).astype(ml_dtypes.bfloat16)
        wo_c = np.zeros((2, P, DIM), ml_dtypes.bfloat16)
        for a in range(2):
            r0 = 64 * (h0 + 2 * a)
            wo_c[a] = WoT[r0:r0 + 128].astype(ml_dtypes.bfloat16)
        in_maps.append({
            "xt": xts,
            "wq": wq_h, "wk": wk_h, "wv": wv_h,
            "bq": np.asarray(bq, np.float32).reshape(1, DIM),
            "bk": np.asarray(bk, np.float32).reshape(1, DIM),
            "bv": np.asarray(bv, np.float32).reshape(1, DIM),
            "wo": wo_c,
        })

    if _trace:
        try:
            import sys
            sys.path.insert(0, "/root/problem/work")
            import ntff_shim
            ntff_shim.install()
        except Exception:
            pass
    res = run_bass_kernel_spmd(nc, in_maps, core_ids=list(range(NCORES)),
                               trace=_trace)
    LAST_EXEC_NS = res.exec_time_ns
    LAST_TRACE = (res.instructions_and_trace[1]
                  if res.instructions_and_trace else None)
    LAST_INSTS = (res.instructions_and_trace[0]
                  if res.instructions_and_trace else None)

    y = np.zeros((B, S, DIM), np.float32)
    for c in range(NCORES):
        y[c // 4] += np.asarray(res.results[c]["yt"], np.float32).T
    y += np.asarray(bo, np.float32)
    return y


# revision 22
# speedup vs baseline: 1.2582x; 1.1188x over previous
"""BigBird attention (faithful .view-split variant) on 8 Trainium2 NeuronCores.

Sharding: the reference's `.reshape(B, H, S, hd)` head-split makes each
(batch, head) attend over a [2048, 64] row-major reshape of a 128-token
chunk's [128, 1024] projection. The 2*16 = 32 (b,h) pairs are sharded 4 per
core (batch x head parallel). The output projection is computed per-core as
a partial sum over its 4 heads (row-parallel over Wo), partials are summed
on the host.

Everything runs in bf16 on the PE (fp32 would double-pump the array).
Chunks are processed in PAIRS (A, B): their q^T/k^T share one [128, 2048]
SBUF tile (A on partitions 0:64, B on 64:128) produced by a single packed
DRAM bounce + DMA-transpose, and the block-sparse QK^T matmuls for A and B
run concurrently on the PE via row-tiling (tile_position row groups 0/64).
Scores go through one exp per strip covering both chunks; AV accumulates
O~^T (+softmax sums via a ones column on V) per 512-query psum bank.
Normalization is all on-chip: 1-lane reciprocal of the sums row, then a
K=1 PE matmul broadcasts 1/s across 64 partitions; V strips are filled by
SBUF->SBUF DMA from the projection output (no DRAM round trip).

The block mask (band + global cols 0/31 + 3 random blocks) is known at
trace time from src_blocks/tgt_blocks, so the sparsity plan is specialized
per call.
"""

import numpy as np
import ml_dtypes

import concourse.bass as bass
import concourse.mybir as mybir
import concourse.tile as tile
from concourse import bacc
from concourse.bass_utils import run_bass_kernel_spmd

B, S, DIM = 2, 2048, 1024
NHEADS, HD, BLK = 16, 64, 64
NB = S // BLK          # 32 block rows/cols
NCORES = 8
HPC = NHEADS * B // NCORES  # 4 chunks (b,h) per core
P = 128
QT = 512               # query columns per psum bank

f32 = mybir.dt.float32
bf16 = mybir.dt.bfloat16

LAST_EXEC_NS = None
LAST_TRACE = None
LAST_INSTS = None


def _block_mask(src_blocks, tgt_blocks):
    i = np.arange(NB)[:, None]
    j = np.arange(NB)[None, :]
    bm = (np.abs(i - j) <= 1) | (j == 0) | (j == NB - 1)
    bm[np.asarray(src_blocks), np.asarray(tgt_blocks)] = True
    return bm


def _plan_strips(bm):
    """Cover the active blocks with k-stacked strips.

    Strip = dict(k=[kb...] (1 or 2 k-blocks stacked on partitions),
                 q0, qn (q-block run), act [len(k), qn] bool, kind).
    Active cells are claimed exactly once across strips so softmax sums
    are exact.  Global columns 0/31 are handled separately (per q-tile).
    """
    claimed = np.zeros((NB, NB), bool)
    strips = []
    claimed[:, 0] = True
    claimed[:, NB - 1] = True
    # band strips: k-pair (2m-1, 2m), q-blocks [2m-2, 2m+2)
    for m in range(1, NB // 2):
        kbs = [2 * m - 1, 2 * m]
        q0, qn = 2 * m - 2, 4
        act = np.zeros((2, qn), bool)
        for ki, k in enumerate(kbs):
            for qi in range(qn):
                q = q0 + qi
                if bm[q, k] and not claimed[q, k]:
                    act[ki, qi] = True
                    claimed[q, k] = True
        strips.append(dict(k=kbs, q0=q0, qn=qn, act=act, kind="band"))
    # leftover random blocks
    rem = np.argwhere(bm & ~claimed)
    byk = {}
    for q, k in rem:
        byk.setdefault(int(k), []).append(int(q))
    for k, qs in sorted(byk.items()):
        qs = sorted(qs)
        while qs:
            q0 = min(max(qs[0] - 1, 0), NB - 4)
            qn = 4
            act = np.zeros((1, qn), bool)
            rest = []
            for q in qs:
                if q0 <= q < q0 + qn:
                    act[0, q - q0] = True
                    claimed[q, k] = True
                else:
                    rest.append(q)
            qs = rest
            strips.append(dict(k=[k], q0=q0, qn=qn, act=act, kind="extra"))
    strips.sort(key=lambda st: st["q0"])
    return strips


def _build_program(strips, use_bias=True):
    nc = bacc.Bacc("TRN2", target_bir_lowering=False, debug=False,
                   num_devices=NCORES)

    d_xt = nc.dram_tensor("xt", [HPC, P, 8 * P], bf16, kind="ExternalInput")
    d_wq = nc.dram_tensor("wq", [P, 8 * DIM], bf16, kind="ExternalInput")
    d_wk = nc.dram_tensor("wk", [P, 8 * DIM], bf16, kind="ExternalInput")
    d_wv = nc.dram_tensor("wv", [P, 8 * DIM], bf16, kind="ExternalInput")
    d_bq = nc.dram_tensor("bq", [1, DIM], f32, kind="ExternalInput")
    d_bk = nc.dram_tensor("bk", [1, DIM], f32, kind="ExternalInput")
    d_bv = nc.dram_tensor("bv", [1, DIM], f32, kind="ExternalInput")
    d_wo = nc.dram_tensor("wo", [2, P, DIM], bf16, kind="ExternalInput")
    d_yt = nc.dram_tensor("yt", [DIM, S], bf16, kind="ExternalOutput")

    with tile.TileContext(nc) as tc:
        _emit(nc, tc, strips, d_xt, (d_wq, d_wk, d_wv),
              (d_bq, d_bk, d_bv), d_wo, d_yt, use_bias)
    nc.compile()
    return nc


def _bank_pieces(strips):
    """Total AV pieces per 512-query psum bank (glob contributes 1 each)."""
    npieces = [1] * 4
    for st in strips:
        q = st["q0"] * BLK
        qhi = (st["q0"] + st["qn"]) * BLK
        while q < qhi:
            u = q // QT
            qe = min(qhi, (u + 1) * QT)
            npieces[u] += 1
            q = qe
    return npieces


def _emit(nc, tc, strips, d_xt, d_w, d_b, d_wo, d_yt, use_bias):
    from contextlib import ExitStack
    Exp = mybir.ActivationFunctionType.Exp
    with ExitStack() as ctx:
        # PSUM: 2 (proj/out) + 3 (O^T accum) + 3 (scores/broadcast) = 8 banks
        psA = ctx.enter_context(tc.tile_pool(name="psA", bufs=2, space="PSUM"))
        psOT = ctx.enter_context(tc.tile_pool(name="psOT", bufs=4,
                                              space="PSUM"))
        psS = ctx.enter_context(tc.tile_pool(name="psS", bufs=1, space="PSUM"))
        dram = ctx.enter_context(tc.tile_pool(name="dram", bufs=1,
                                              space="DRAM"))
        sbB = ctx.enter_context(tc.tile_pool(name="sbB", bufs=1))
        sbW = ctx.enter_context(tc.tile_pool(name="sbW", bufs=2))
        sbE = ctx.enter_context(tc.tile_pool(name="sbE", bufs=3))
        sbN = ctx.enter_context(tc.tile_pool(name="sbN", bufs=2))

        # weights + constants
        wts = {}
        for nm, dw in zip("qkv", d_w):
            w = sbB.tile([P, 8 * DIM], bf16, tag=f"w{nm}", name=f"w{nm}")
            for kt in range(8):
                nc.sync.dma_start(w[:, kt * DIM:(kt + 1) * DIM],
                                  dw[:, kt * DIM:(kt + 1) * DIM])
            wts[nm] = w
        wob = sbB.tile([P, 2 * DIM], bf16, tag="wob")
        nc.sync.dma_start(wob[:, 0:DIM], d_wo[0])
        nc.sync.dma_start(wob[:, DIM:2 * DIM], d_wo[1])
        ones = sbB.tile([P, BLK], bf16, tag="ones")
        nc.vector.memset(ones[:], 1.0)
        bts = {}
        if use_bias:
            for nm, db in zip("qkv", d_b):
                bt = sbB.tile([P, DIM], f32, tag=f"b{nm}", name=f"b{nm}")
                nc.sync.dma_start(bt[:], db[:].to_broadcast((P, DIM)))
                bts[nm] = bt
        xtiles = [sbB.tile([P, 8 * P], bf16, tag=f"xt{i}", name=f"xt{i}")
                  for i in range(HPC)]
        for i in range(HPC):
            nc.sync.dma_start(xtiles[i][:], d_xt[i])

        # o2[a]: head-pair-stacked normalized O^T for phase C
        o2 = [sbB.tile([P, S], bf16, tag=f"o2_{a}", name=f"o2_{a}")
              for a in range(2)]

        npieces = _bank_pieces(strips)

        for pr in range(2):          # chunk pair (2*pr, 2*pr+1)
            # ---------------- Phase A: QKV projections -------------------
            lint = {"q": sbW.tile([P, 2 * DIM], bf16, tag="lq", name=f"lq{pr}"),
                    "k": sbW.tile([P, 2 * DIM], bf16, tag="lk", name=f"lk{pr}")}
            lv = [sbW.tile([P, DIM], bf16, tag=f"lv{h}", name=f"lv{h}{pr}")
                  for h in range(2)]
            for nm in "qkv":
                w = wts[nm]
                for half in range(2):
                    i = 2 * pr + half
                    xt = xtiles[i]
                    for nb2 in range(2):
                        ps = psA.tile([P, 512], f32, tag="mm512")
                        for kt in range(8):
                            nc.tensor.matmul(
                                ps[:],
                                lhsT=xt[:, kt * P:(kt + 1) * P],
                                rhs=w[:, kt * DIM + nb2 * 512:
                                      kt * DIM + nb2 * 512 + 512],
                                start=(kt == 0), stop=(kt == 7))
                        if nm == "v":
                            out_ap = lv[half][:, nb2 * 512:(nb2 + 1) * 512
                                              ].rearrange("p (c d) -> p c d",
                                                          d=64)
                        else:
                            out_ap = lint[nm][:].rearrange(
                                "p (c x) -> p c x",
                                x=P)[:, nb2 * 8:(nb2 + 1) * 8,
                                     half * 64:(half + 1) * 64]
                        src = ps[:].rearrange("p (c d) -> p c d", d=64)
                        if use_bias:
                            nc.vector.tensor_add(
                                out_ap, src,
                                bts[nm][:, nb2 * 512:(nb2 + 1) * 512
                                        ].rearrange("p (c d) -> p c d", d=64))
                        elif nm == "v":
                            # contiguous dst: DVE runs it at full rate
                            nc.vector.tensor_copy(out_ap, src)
                        else:
                            # strided dst is slow on DVE; ACT handles it
                            nc.scalar.copy(out_ap, src)
            # bounce q,k through DRAM to transpose; v stays in SBUF
            qkt = {}
            for nm in "qk":
                dl = dram.tile([S, P], bf16, tag=f"d{nm}{pr}",
                               name=f"d{nm}{pr}")
                nc.sync.dma_start(dl[:], lint[nm][:])
                t = sbW.tile([P, S], bf16, tag=f"{nm}t", name=f"{nm}t{pr}")
                nc.sync.dma_start(t[:], dl[:], transpose=True)
                qkt[nm] = t
            qt_, kt_ = qkt["q"], qkt["k"]

            # V strips: bounce through DRAM (partition-scatter APs are only
            # legal with a DRAM side), global/extra tiles via flat-order
            # SBUF->SBUF (partition-major element zip, like the lint store)
            v2b, v2g, dvs = [], [], []
            for half in range(2):
                dv = dram.tile([P, DIM], bf16, tag=f"dv{half}{pr}",
                               name=f"dv{half}{pr}")
                nc.sync.dma_start(dv[:], lv[half][:])
                vb = sbN.tile([P, 15 * 65], bf16, tag=f"v2b{half}",
                              name=f"v2b{half}{pr}")
                nc.sync.dma_start(
                    vb[:].rearrange("p (g e) -> p g e", e=65)[:, :, 0:64],
                    dv[4:124].rearrange("(g a) (b d) -> (a b) g d",
                                        a=8, d=64))
                nc.vector.memset(
                    vb[:].rearrange("p (g e) -> p g e", e=65)[:, :, 64:65],
                    1.0)
                v2b.append(vb)
                vg = sbN.tile([P, 65], bf16, tag=f"v2g{half}",
                              name=f"v2g{half}{pr}")
                nc.sync.dma_start(
                    vg[0:64, 0:64],
                    dv[0:4].rearrange("t (c d) -> (t c) d", d=64))
                nc.sync.dma_start(
                    vg[64:128, 0:64],
                    dv[124:128].rearrange("t (c d) -> (t c) d", d=64))
                nc.vector.memset(vg[:, 64:65], 1.0)
                v2g.append(vg)
                dvs.append(dv)
            # extra-strip V tiles: A on rows 0:64, B on rows 64:128
            vxs = {}
            for si, st in enumerate(strips):
                if st["kind"] != "extra":
                    continue
                kb = st["k"][0]
                vx = sbN.tile([P, 65], bf16, tag=f"vx{si}", name=f"vx{si}{pr}")
                for half in range(2):
                    nc.sync.dma_start(
                        vx[half * 64:(half + 1) * 64, 0:64],
                        dvs[half][kb * 4:kb * 4 + 4].rearrange(
                            "t (c d) -> (t c) d", d=64))
                nc.vector.memset(vx[:, 64:65], 1.0)
                vxs[si] = vx

            # ---------------- Phase B: attention, q-tile major -----------
            # Concurrent row-tiled matmuls (different tile_position rows)
            # must NOT write the same PSUM bank at the same partitions —
            # that faults on HW.  Chunk A scores go to bank 0 of a 2-bank
            # [128, 1024] tile, chunk B to bank 1; one exp covers both.
            ots = {}          # (half, u) -> psum tile [65, 512]
            done = {}
            egs = {}          # u -> eg sbuf tile (glob E, both halves)

            def get_ot(half, u):
                # Banks must be OPENED by a full-width start piece so the
                # accumulate-vs-overwrite state stays uniform per bank: the
                # global-columns AV piece (all 512 q) plays that role.
                if (half, u) not in ots:
                    if u not in egs:
                        emit_glob_scores(u)
                    eg = egs[u]
                    ot = psOT.tile([65, QT], f32, tag="ot",
                                   name=f"ot{pr}{half}{u}")
                    ots[(half, u)] = ot
                    done[(half, u)] = 0
                    nc.tensor.matmul(
                        ot[0:65, :], lhsT=v2g[half][:],
                        rhs=eg[:, half * QT:(half + 1) * QT],
                        start=True, stop=(npieces[u] == 1))
                    done[(half, u)] = 1
                    if done[(half, u)] == npieces[u]:
                        finish(half, u)
                return ots[(half, u)]

            def emit_glob_scores(u):
                pg = psS.tile([P, 2 * QT], f32, tag="s", name=f"g{u}")
                eg = sbE.tile([P, 2 * QT], bf16, tag="eg", name=f"eg{u}")
                for half in range(2):
                    h64 = half * 64
                    off = half * QT
                    qcols = qt_[h64:h64 + 64, u * QT:(u + 1) * QT]
                    nc.tensor.matmul(pg[0:64, off:off + QT],
                                     lhsT=kt_[h64:h64 + 64, 0:64],
                                     rhs=qcols, start=True, stop=True)
                    nc.tensor.matmul(pg[64:128, off:off + QT],
                                     lhsT=kt_[h64:h64 + 64, S - 64:S],
                                     rhs=qcols, start=True, stop=True)
                nc.scalar.activation(eg[:], pg[:], Exp, scale=0.125)
                egs[u] = eg

            def finish(half, u):
                # normalize O~^T by the sums row, write into o2[pr]
                import os
                ot = ots.pop((half, u))
                if os.environ.get("BB_NORM_DRAM"):
                    # baseline-style: bounce sums through DRAM to reshape and
                    # broadcast across partitions
                    srow = sbN.tile([65, QT], f32, tag="srow",
                                    name=f"sr{half}{u}")
                    nc.scalar.copy(srow[64:65, :], ot[64:65, :])
                    dsum = dram.tile([1, QT], f32, tag=f"ds{half}{u}",
                                     name=f"ds{half}{u}")
                    nc.sync.dma_start(dsum[:], srow[64:65, :])
                    ssum = sbN.tile([P, 4], f32, tag="ssum",
                                    name=f"ss{half}{u}")
                    nc.sync.dma_start(
                        ssum[:], dsum[:].rearrange("o (p f) -> (o p) f", f=4))
                    rr4 = sbN.tile([P, 4], f32, tag="rr4", name=f"r4{half}{u}")
                    nc.vector.reciprocal(rr4[:], ssum[:])
                    drr = dram.tile([1, QT], f32, tag=f"dr{half}{u}",
                                    name=f"dr{half}{u}")
                    nc.sync.dma_start(
                        drr[:].rearrange("o (p f) -> (o p) f", f=4), rr4[:])
                    rbc = sbN.tile([64, QT], f32, tag="rbc",
                                   name=f"rbc{half}{u}")
                    nc.sync.dma_start(rbc[:], drr[:].to_broadcast((64, QT)))
                else:
                    # broadcast the sums row via a K=1 matmul, then take the
                    # reciprocal on 64 lanes (a [1,512] DVE op would be
                    # single-lane and ~3.3us)
                    srow = sbN.tile([65, QT], bf16, tag="rr",
                                    name=f"rr{half}{u}")
                    nc.scalar.copy(srow[64:65, :], ot[64:65, :])
                    rbp = psA.tile([P, QT], f32, tag="mm512",
                                   name=f"rbp{half}{u}")
                    nc.tensor.matmul(rbp[0:64, :], lhsT=ones[64:65, 0:64],
                                     rhs=srow[64:65, :], start=True,
                                     stop=True)
                    rbc = sbN.tile([64, QT], f32, tag="rbc",
                                   name=f"rbc{half}{u}")
                    nc.vector.reciprocal(rbc[:], rbp[0:64, :])
                cols = slice(u * QT, (u + 1) * QT)
                if half == 0:
                    nc.vector.tensor_mul(o2[pr][0:64, cols],
                                         ot[0:64, :], rbc[:])
                else:
                    o2t = sbN.tile([64, QT], bf16, tag="o2t",
                                   name=f"o2t{half}{u}")
                    nc.vector.tensor_mul(o2t[:], ot[0:64, :], rbc[:])
                    nc.sync.dma_start(o2[pr][64:128, cols], o2t[:])

            def av_pieces(st_q0, st_qn, lhs, et, ecol, erow, rows):
                # accumulate AV pieces into the per-u psum banks (both halves
                # share the strip's E tile; `erow`/`ecol` locate each half's
                # scores inside it)
                for half in range(2):
                    qlo = st_q0 * BLK
                    qhi = (st_q0 + st_qn) * BLK
                    q = qlo
                    while q < qhi:
                        u = q // QT
                        qe = min(qhi, (u + 1) * QT)
                        ot = get_ot(half, u)
                        nc.tensor.matmul(
                            ot[0:65, q - u * QT:qe - u * QT],
                            lhsT=lhs[half],
                            rhs=et[erow[half]:erow[half] + rows,
                                   ecol[half] + q - qlo:ecol[half] + qe - qlo],
                            start=(done[(half, u)] == 0),
                            stop=(done[(half, u)] == npieces[u] - 1))
                        done[(half, u)] += 1
                        if done[(half, u)] == npieces[u]:
                            finish(half, u)
                        q = qe

            strips_by_u = [[] for _ in range(4)]
            for si, st in enumerate(strips):
                strips_by_u[st["q0"] * BLK // QT].append((si, st))

            for u in range(4):
                # open this q-tile's banks (emits the glob pieces)
                for half in range(2):
                    get_ot(half, u)

                # --- band strips, paired two per 2-bank scores tile ---
                bands = [s for s in strips_by_u[u] if s[1]["kind"] == "band"]
                extras = [s for s in strips_by_u[u] if s[1]["kind"] != "band"]
                for gi in range(0, len(bands), 2):
                    grp = bands[gi:gi + 2]
                    pss = psS.tile([P, 2 * QT], f32, tag="s",
                                   name=f"s{grp[0][0]}")
                    em = sbE.tile([P, 2 * QT], bf16, tag="em",
                                  name=f"em{grp[0][0]}")
                    for sl, (si, st) in enumerate(grp):
                        qlo, qn = st["q0"] * BLK, st["qn"] * BLK
                        k0 = st["k"][0] * BLK
                        for half in range(2):
                            h64 = half * 64
                            c0 = half * QT + sl * 256
                            nc.tensor.matmul(
                                pss[:, c0:c0 + qn],
                                lhsT=kt_[h64:h64 + 64, k0:k0 + 128],
                                rhs=qt_[h64:h64 + 64, qlo:qlo + qn],
                                start=True, stop=True)
                    wcols = 256 * len(grp)
                    nc.scalar.activation(em[:, 0:wcols], pss[:, 0:wcols],
                                         Exp, scale=0.125)
                    nc.scalar.activation(em[:, QT:QT + wcols],
                                         pss[:, QT:QT + wcols],
                                         Exp, scale=0.125)
                    for sl, (si, st) in enumerate(grp):
                        for half in range(2):
                            for ki in range(2):
                                for qi in range(st["qn"]):
                                    if not st["act"][ki, qi]:
                                        c0 = half * QT + sl * 256 + qi * 64
                                        nc.gpsimd.memset(
                                            em[ki * 64:(ki + 1) * 64,
                                               c0:c0 + 64], 0.0)
                        g = (st["k"][0] - 1) // 2
                        lhs = [v2b[h][:, g * 65:(g + 1) * 65]
                               for h in range(2)]
                        av_pieces(st["q0"], st["qn"], lhs, em,
                                  ecol=(sl * 256, QT + sl * 256),
                                  erow=(0, 0), rows=128)

                # --- extra strips (single k-block), A and B quadrants of
                # one bank at disjoint partition ranges ---
                for si, st in extras:
                    qlo, qn = st["q0"] * BLK, st["qn"] * BLK
                    kb = st["k"][0]
                    psx = psS.tile([P, 2 * QT], f32, tag="s", name=f"s{si}")
                    ex = sbE.tile([P, 512], bf16, tag="ex", name=f"ex{si}")
                    for half in range(2):
                        h64 = half * 64
                        nc.gpsimd.memset(
                            ex[h64:h64 + 64,
                               half * 256:half * 256 + qn], 0.0)
                        nc.tensor.matmul(
                            psx[h64:h64 + 64,
                                half * 256:half * 256 + qn],
                            lhsT=kt_[h64:h64 + 64,
                                     kb * BLK:kb * BLK + 64],
                            rhs=qt_[h64:h64 + 64, qlo:qlo + qn],
                            start=True, stop=True)
                        for qi in range(st["qn"]):
                            if st["act"][0, qi]:
                                nc.scalar.activation(
                                    ex[h64:h64 + 64,
                                       half * 256 + qi * 64:
                                       half * 256 + (qi + 1) * 64],
                                    psx[h64:h64 + 64,
                                        half * 256 + qi * 64:
                                        half * 256 + (qi + 1) * 64],
                                    Exp, scale=0.125)
                    lhs = [vxs[si][0:64, :], vxs[si][64:128, :]]
                    av_pieces(st["q0"], st["qn"], lhs, ex,
                              ecol=(0, 256), erow=(0, 64), rows=64)

        # ---------------- Phase C: partial output projection -------------
        with tc.tile_pool(name="yp", bufs=3) as yp:
            for qb in range(4):
                for mt in range(8):
                    ps = psA.tile([P, 512], f32, tag="mm512")
                    for a in range(2):
                        nc.tensor.matmul(
                            ps[:],
                            lhsT=wob[:, a * DIM + mt * P:
                                     a * DIM + (mt + 1) * P],
                            rhs=o2[a][:, qb * 512:(qb + 1) * 512],
                            start=(a == 0), stop=(a == 1))
                    yt = yp.tile([P, 512], bf16, tag="yt")
                    nc.vector.tensor_copy(yt[:], ps[:])
                    nc.sync.dma_start(
                        d_yt[mt * P:(mt + 1) * P, qb * 512:(qb + 1) * 512],
                        yt[:])


def kernel(x, Wq, bq, Wk, bk, Wv, bv, Wo, bo, src_blocks, tgt_blocks,
           _trace=False):
    global LAST_EXEC_NS, LAST_TRACE, LAST_INSTS
    x = np.asarray(x, np.float32)
    bm = _block_mask(np.asarray(src_blocks), np.asarray(tgt_blocks))
    strips = _plan_strips(bm)
    use_bias = bool(np.any(np.asarray(bq)) or np.any(np.asarray(bk))
                    or np.any(np.asarray(bv)))
    nc = _build_program(strips, use_bias)

    # host-side shard prep
    # W layout for rhs: w[p, kt*1024 + j] = W[j, kt*128 + p]
    def w_rhs(W):
        Wt = np.ascontiguousarray(np.asarray(W, np.float32).T)  # [in, out]
        return np.ascontiguousarray(
            Wt.reshape(8, P, DIM).transpose(1, 0, 2).reshape(P, 8 * DIM)
        ).astype(ml_dtypes.bfloat16)

    wq_h, wk_h, wv_h = w_rhs(Wq), w_rhs(Wk), w_rhs(Wv)
    WoT = np.asarray(Wo, np.float32).T  # [in(=64*head), out]
    x4 = x.reshape(B, NHEADS, P, DIM)

    in_maps = []
    for c in range(NCORES):
        b = c // 4
        h0 = 4 * (c % 4)
        xc = x4[b, h0:h0 + 4]                       # [4, 128, 1024]
        xt = np.ascontiguousarray(xc.transpose(0, 2, 1))  # [4, 1024, 128]
        # xt dram layout [4, 128, 8*128]: xts[i, p, kt*128+t] = x[t, kt*128+p]
        xts = np.ascontiguousarray(
            xt.reshape(HPC, 8, P, P).transpose(0, 2, 1, 3).reshape(
                HPC, P, 8 * P)).astype(ml_dtypes.bfloat16)
        wo_c = np.zeros((2, P, DIM), ml_dtypes.bfloat16)
        for a in range(2):
            r0 = 64 * (h0 + 2 * a)
            wo_c[a] = WoT[r0:r0 + 128].astype(ml_dtypes.bfloat16)
        in_maps.append({
            "xt": xts,
            "wq": wq_h, "wk": wk_h, "wv": wv_h,
            "bq": np.asarray(bq, np.float32).reshape(1, DIM),
            "bk": np.asarray(bk, np.float32).reshape(1, DIM),
            "bv": np.asarray(bv, np.float32).reshape(1, DIM),
            "wo": wo_c,
        })

    if _trace:
        try:
            import sys
            sys.path.insert(0, "/root/problem/work")
            import ntff_shim
            ntff_shim.install()
        except Exception:
            pass
    res = run_bass_kernel_spmd(nc, in_maps, core_ids=list(range(NCORES)),
                               trace=_trace)
    LAST_EXEC_NS = res.exec_time_ns
    LAST_TRACE = (res.instructions_and_trace[1]
                  if res.instructions_and_trace else None)
    LAST_INSTS = (res.instructions_and_trace[0]
                  if res.instructions_and_trace else None)

    y = np.zeros((B, S, DIM), np.float32)
    for c in range(NCORES):
        y[c // 4] += np.asarray(res.results[c]["yt"], np.float32).T
    y += np.asarray(bo, np.float32)
    return y


# revision 23
# speedup vs baseline: 1.3323x; 1.0589x over previous
"""BigBird attention (faithful .view-split variant) on 8 Trainium2 NeuronCores.

Sharding: the reference's `.reshape(B, H, S, hd)` head-split makes each
(batch, head) attend over a [2048, 64] row-major reshape of a 128-token
chunk's [128, 1024] projection. The 2*16 = 32 (b,h) pairs are sharded 4 per
core (batch x head parallel). The output projection is computed per-core as
a partial sum over its 4 heads (row-parallel over Wo), partials are summed
on the host.

Everything runs in bf16 on the PE (fp32 would double-pump the array).
Chunks are processed in PAIRS (A, B): their q^T/k^T share one [128, 2048]
SBUF tile (A on partitions 0:64, B on 64:128) produced by a single packed
DRAM bounce + DMA-transpose, and the block-sparse QK^T matmuls for A and B
run concurrently on the PE via row-tiling (tile_position row groups 0/64).
Scores go through one exp per strip covering both chunks; AV accumulates
O~^T (+softmax sums via a ones column on V) per 512-query psum bank.
Normalization is all on-chip: 1-lane reciprocal of the sums row, then a
K=1 PE matmul broadcasts 1/s across 64 partitions; V strips are filled by
SBUF->SBUF DMA from the projection output (no DRAM round trip).

The block mask (band + global cols 0/31 + 3 random blocks) is known at
trace time from src_blocks/tgt_blocks, so the sparsity plan is specialized
per call.
"""

import numpy as np
import ml_dtypes

import concourse.bass as bass
import concourse.mybir as mybir
import concourse.tile as tile
from concourse import bacc
from concourse.bass_utils import run_bass_kernel_spmd

B, S, DIM = 2, 2048, 1024
NHEADS, HD, BLK = 16, 64, 64
NB = S // BLK          # 32 block rows/cols
NCORES = 8
HPC = NHEADS * B // NCORES  # 4 chunks (b,h) per core
P = 128
QT = 512               # query columns per psum bank

f32 = mybir.dt.float32
bf16 = mybir.dt.bfloat16

LAST_EXEC_NS = None
LAST_TRACE = None
LAST_INSTS = None


def _block_mask(src_blocks, tgt_blocks):
    i = np.arange(NB)[:, None]
    j = np.arange(NB)[None, :]
    bm = (np.abs(i - j) <= 1) | (j == 0) | (j == NB - 1)
    bm[np.asarray(src_blocks), np.asarray(tgt_blocks)] = True
    return bm


def _plan_strips(bm):
    """Cover the active blocks with k-stacked strips.

    Strip = dict(k=[kb...] (1 or 2 k-blocks stacked on partitions),
                 q0, qn (q-block run), act [len(k), qn] bool, kind).
    Active cells are claimed exactly once across strips so softmax sums
    are exact.  Global columns 0/31 are handled separately (per q-tile).
    """
    claimed = np.zeros((NB, NB), bool)
    strips = []
    claimed[:, 0] = True
    claimed[:, NB - 1] = True
    # band strips: k-pair (2m-1, 2m), q-blocks [2m-2, 2m+2)
    for m in range(1, NB // 2):
        kbs = [2 * m - 1, 2 * m]
        q0, qn = 2 * m - 2, 4
        act = np.zeros((2, qn), bool)
        for ki, k in enumerate(kbs):
            for qi in range(qn):
                q = q0 + qi
                if bm[q, k] and not claimed[q, k]:
                    act[ki, qi] = True
                    claimed[q, k] = True
        strips.append(dict(k=kbs, q0=q0, qn=qn, act=act, kind="band"))
    # leftover random blocks
    rem = np.argwhere(bm & ~claimed)
    byk = {}
    for q, k in rem:
        byk.setdefault(int(k), []).append(int(q))
    for k, qs in sorted(byk.items()):
        qs = sorted(qs)
        while qs:
            q0 = min(max(qs[0] - 1, 0), NB - 4)
            qn = 4
            act = np.zeros((1, qn), bool)
            rest = []
            for q in qs:
                if q0 <= q < q0 + qn:
                    act[0, q - q0] = True
                    claimed[q, k] = True
                else:
                    rest.append(q)
            qs = rest
            strips.append(dict(k=[k], q0=q0, qn=qn, act=act, kind="extra"))
    strips.sort(key=lambda st: st["q0"])
    return strips


def _build_program(strips, use_bias=True):
    nc = bacc.Bacc("TRN2", target_bir_lowering=False, debug=False,
                   num_devices=NCORES)

    d_xt = nc.dram_tensor("xt", [HPC, P, 8 * P], bf16, kind="ExternalInput")
    d_wq = nc.dram_tensor("wq", [P, 8 * DIM], bf16, kind="ExternalInput")
    d_wk = nc.dram_tensor("wk", [P, 8 * DIM], bf16, kind="ExternalInput")
    d_wv = nc.dram_tensor("wv", [P, 8 * DIM], bf16, kind="ExternalInput")
    d_bq = nc.dram_tensor("bq", [1, DIM], f32, kind="ExternalInput")
    d_bk = nc.dram_tensor("bk", [1, DIM], f32, kind="ExternalInput")
    d_bv = nc.dram_tensor("bv", [1, DIM], f32, kind="ExternalInput")
    d_wo = nc.dram_tensor("wo", [2, P, DIM], bf16, kind="ExternalInput")
    d_yt = nc.dram_tensor("yt", [DIM, S], bf16, kind="ExternalOutput")

    with tile.TileContext(nc) as tc:
        _emit(nc, tc, strips, d_xt, (d_wq, d_wk, d_wv),
              (d_bq, d_bk, d_bv), d_wo, d_yt, use_bias)
    nc.compile()
    return nc


def _bank_pieces(strips):
    """Total AV pieces per 512-query psum bank (glob contributes 1 each)."""
    npieces = [1] * 4
    for st in strips:
        q = st["q0"] * BLK
        qhi = (st["q0"] + st["qn"]) * BLK
        while q < qhi:
            u = q // QT
            qe = min(qhi, (u + 1) * QT)
            npieces[u] += 1
            q = qe
    return npieces


def _emit(nc, tc, strips, d_xt, d_w, d_b, d_wo, d_yt, use_bias):
    from contextlib import ExitStack
    Exp = mybir.ActivationFunctionType.Exp
    with ExitStack() as ctx:
        # PSUM: 2 (proj/out) + 3 (O^T accum) + 3 (scores/broadcast) = 8 banks
        psA = ctx.enter_context(tc.tile_pool(name="psA", bufs=2, space="PSUM"))
        psOT = ctx.enter_context(tc.tile_pool(name="psOT", bufs=4,
                                              space="PSUM"))
        psS = ctx.enter_context(tc.tile_pool(name="psS", bufs=1, space="PSUM"))
        dram = ctx.enter_context(tc.tile_pool(name="dram", bufs=1,
                                              space="DRAM"))
        sbB = ctx.enter_context(tc.tile_pool(name="sbB", bufs=1))
        sbW = ctx.enter_context(tc.tile_pool(name="sbW", bufs=2))
        sbE = ctx.enter_context(tc.tile_pool(name="sbE", bufs=3))
        sbN = ctx.enter_context(tc.tile_pool(name="sbN", bufs=2))

        # weights + constants
        wts = {}
        for nm, dw in zip("qkv", d_w):
            w = sbB.tile([P, 8 * DIM], bf16, tag=f"w{nm}", name=f"w{nm}")
            for kt in range(8):
                nc.sync.dma_start(w[:, kt * DIM:(kt + 1) * DIM],
                                  dw[:, kt * DIM:(kt + 1) * DIM])
            wts[nm] = w
        wob = sbB.tile([P, 2 * DIM], bf16, tag="wob")
        nc.sync.dma_start(wob[:, 0:DIM], d_wo[0])
        nc.sync.dma_start(wob[:, DIM:2 * DIM], d_wo[1])
        ones = sbB.tile([P, BLK], bf16, tag="ones")
        nc.vector.memset(ones[:], 1.0)
        bts = {}
        if use_bias:
            for nm, db in zip("qkv", d_b):
                bt = sbB.tile([P, DIM], f32, tag=f"b{nm}", name=f"b{nm}")
                nc.sync.dma_start(bt[:], db[:].to_broadcast((P, DIM)))
                bts[nm] = bt
        xtiles = [sbB.tile([P, 8 * P], bf16, tag=f"xt{i}", name=f"xt{i}")
                  for i in range(HPC)]
        for i in range(HPC):
            nc.sync.dma_start(xtiles[i][:], d_xt[i])

        # o2[a]: head-pair-stacked normalized O^T for phase C
        o2 = [sbB.tile([P, S], bf16, tag=f"o2_{a}", name=f"o2_{a}")
              for a in range(2)]

        npieces = _bank_pieces(strips)

        for pr in range(2):          # chunk pair (2*pr, 2*pr+1)
            # ---------------- Phase A: QKV projections -------------------
            lint = {"q": sbW.tile([P, 2 * DIM], bf16, tag="lq", name=f"lq{pr}"),
                    "k": sbW.tile([P, 2 * DIM], bf16, tag="lk", name=f"lk{pr}")}
            lv = [sbW.tile([P, DIM], bf16, tag=f"lv{h}", name=f"lv{h}{pr}")
                  for h in range(2)]
            for nm in "qkv":
                w = wts[nm]
                for half in range(2):
                    i = 2 * pr + half
                    xt = xtiles[i]
                    for nb2 in range(2):
                        ps = psA.tile([P, 512], f32, tag="mm512")
                        for kt in range(8):
                            nc.tensor.matmul(
                                ps[:],
                                lhsT=xt[:, kt * P:(kt + 1) * P],
                                rhs=w[:, kt * DIM + nb2 * 512:
                                      kt * DIM + nb2 * 512 + 512],
                                start=(kt == 0), stop=(kt == 7))
                        if nm == "v":
                            out_ap = lv[half][:, nb2 * 512:(nb2 + 1) * 512
                                              ].rearrange("p (c d) -> p c d",
                                                          d=64)
                        else:
                            out_ap = lint[nm][:].rearrange(
                                "p (c x) -> p c x",
                                x=P)[:, nb2 * 8:(nb2 + 1) * 8,
                                     half * 64:(half + 1) * 64]
                        src = ps[:].rearrange("p (c d) -> p c d", d=64)
                        if use_bias:
                            nc.vector.tensor_add(
                                out_ap, src,
                                bts[nm][:, nb2 * 512:(nb2 + 1) * 512
                                        ].rearrange("p (c d) -> p c d", d=64))
                        elif nm == "v":
                            # contiguous dst: DVE runs it at full rate
                            nc.vector.tensor_copy(out_ap, src)
                        else:
                            # strided dst is slow on DVE; ACT handles it
                            nc.scalar.copy(out_ap, src)
            # bounce q,k through DRAM to transpose; v stays in SBUF
            qkt = {}
            for nm in "qk":
                dl = dram.tile([S, P], bf16, tag=f"d{nm}{pr}",
                               name=f"d{nm}{pr}")
                nc.sync.dma_start(dl[:], lint[nm][:])
                t = sbW.tile([P, S], bf16, tag=f"{nm}t", name=f"{nm}t{pr}")
                nc.sync.dma_start(t[:], dl[:], transpose=True)
                qkt[nm] = t
            qt_, kt_ = qkt["q"], qkt["k"]

            # V strips: bounce through DRAM (partition-scatter APs are only
            # legal with a DRAM side), global/extra tiles via flat-order
            # SBUF->SBUF (partition-major element zip, like the lint store)
            v2b, v2g, dvs = [], [], []
            for half in range(2):
                dv = dram.tile([P, DIM], bf16, tag=f"dv{half}{pr}",
                               name=f"dv{half}{pr}")
                nc.sync.dma_start(dv[:], lv[half][:])
                vb = sbN.tile([P, 15 * 65], bf16, tag=f"v2b{half}",
                              name=f"v2b{half}{pr}")
                nc.sync.dma_start(
                    vb[:].rearrange("p (g e) -> p g e", e=65)[:, :, 0:64],
                    dv[4:124].rearrange("(g a) (b d) -> (a b) g d",
                                        a=8, d=64))
                nc.vector.memset(
                    vb[:].rearrange("p (g e) -> p g e", e=65)[:, :, 64:65],
                    1.0)
                v2b.append(vb)
                vg = sbN.tile([P, 65], bf16, tag=f"v2g{half}",
                              name=f"v2g{half}{pr}")
                nc.sync.dma_start(
                    vg[0:64, 0:64],
                    dv[0:4].rearrange("t (c d) -> (t c) d", d=64))
                nc.sync.dma_start(
                    vg[64:128, 0:64],
                    dv[124:128].rearrange("t (c d) -> (t c) d", d=64))
                nc.vector.memset(vg[:, 64:65], 1.0)
                v2g.append(vg)
                dvs.append(dv)
            # extra-strip V tiles: A on rows 0:64, B on rows 64:128
            vxs = {}
            for si, st in enumerate(strips):
                if st["kind"] != "extra":
                    continue
                kb = st["k"][0]
                vx = sbN.tile([P, 65], bf16, tag=f"vx{si}", name=f"vx{si}{pr}")
                for half in range(2):
                    nc.sync.dma_start(
                        vx[half * 64:(half + 1) * 64, 0:64],
                        dvs[half][kb * 4:kb * 4 + 4].rearrange(
                            "t (c d) -> (t c) d", d=64))
                nc.vector.memset(vx[:, 64:65], 1.0)
                vxs[si] = vx

            # ---------------- Phase B: attention, q-tile major -----------
            # Concurrent row-tiled matmuls (different tile_position rows)
            # must NOT write the same PSUM bank at the same partitions —
            # that faults on HW.  Chunk A scores go to bank 0 of a 2-bank
            # [128, 1024] tile, chunk B to bank 1; one exp covers both.
            ots = {}          # (half, u) -> psum tile [65, 512]
            done = {}
            egs = {}          # u -> eg sbuf tile (glob E, both halves)

            def get_ot(half, u):
                # Banks must be OPENED by a full-width start piece so the
                # accumulate-vs-overwrite state stays uniform per bank: the
                # global-columns AV piece (all 512 q) plays that role.
                if (half, u) not in ots:
                    if u not in egs:
                        emit_glob_scores(u)
                    eg = egs[u]
                    ot = psOT.tile([65, QT], f32, tag="ot",
                                   name=f"ot{pr}{half}{u}")
                    ots[(half, u)] = ot
                    done[(half, u)] = 0
                    nc.tensor.matmul(
                        ot[0:65, :], lhsT=v2g[half][:],
                        rhs=eg[:, half * QT:(half + 1) * QT],
                        start=True, stop=(npieces[u] == 1))
                    done[(half, u)] = 1
                    if done[(half, u)] == npieces[u]:
                        finish(half, u)
                return ots[(half, u)]

            def emit_glob_scores(u):
                pg = psS.tile([P, 2 * QT], f32, tag="s", name=f"g{u}")
                eg = sbE.tile([P, 2 * QT], bf16, tag="eg", name=f"eg{u}")
                for half in range(2):
                    h64 = half * 64
                    off = half * QT
                    qcols = qt_[h64:h64 + 64, u * QT:(u + 1) * QT]
                    nc.tensor.matmul(pg[0:64, off:off + QT],
                                     lhsT=kt_[h64:h64 + 64, 0:64],
                                     rhs=qcols, start=True, stop=True)
                    nc.tensor.matmul(pg[64:128, off:off + QT],
                                     lhsT=kt_[h64:h64 + 64, S - 64:S],
                                     rhs=qcols, start=True, stop=True)
                nc.scalar.activation(eg[:], pg[:], Exp, scale=0.125)
                egs[u] = eg

            def finish(half, u):
                # normalize O~^T by the sums row, write into o2[pr]
                import os
                ot = ots.pop((half, u))
                if os.environ.get("BB_NORM_DRAM"):
                    # baseline-style: bounce sums through DRAM to reshape and
                    # broadcast across partitions
                    srow = sbN.tile([65, QT], f32, tag="srow",
                                    name=f"sr{half}{u}")
                    nc.scalar.copy(srow[64:65, :], ot[64:65, :])
                    dsum = dram.tile([1, QT], f32, tag=f"ds{half}{u}",
                                     name=f"ds{half}{u}")
                    nc.sync.dma_start(dsum[:], srow[64:65, :])
                    ssum = sbN.tile([P, 4], f32, tag="ssum",
                                    name=f"ss{half}{u}")
                    nc.sync.dma_start(
                        ssum[:], dsum[:].rearrange("o (p f) -> (o p) f", f=4))
                    rr4 = sbN.tile([P, 4], f32, tag="rr4", name=f"r4{half}{u}")
                    nc.vector.reciprocal(rr4[:], ssum[:])
                    drr = dram.tile([1, QT], f32, tag=f"dr{half}{u}",
                                    name=f"dr{half}{u}")
                    nc.sync.dma_start(
                        drr[:].rearrange("o (p f) -> (o p) f", f=4), rr4[:])
                    rbc = sbN.tile([64, QT], f32, tag="rbc",
                                   name=f"rbc{half}{u}")
                    nc.sync.dma_start(rbc[:], drr[:].to_broadcast((64, QT)))
                else:
                    # broadcast the sums row via a K=1 matmul, then take the
                    # reciprocal on 64 lanes (a [1,512] DVE op would be
                    # single-lane and ~3.3us)
                    srow = sbN.tile([65, QT], bf16, tag="rr",
                                    name=f"rr{half}{u}")
                    nc.scalar.copy(srow[64:65, :], ot[64:65, :])
                    rbp = psA.tile([P, QT], f32, tag="mm512",
                                   name=f"rbp{half}{u}")
                    nc.tensor.matmul(rbp[0:64, :], lhsT=ones[64:65, 0:64],
                                     rhs=srow[64:65, :], start=True,
                                     stop=True)
                    rbc = sbN.tile([64, QT], f32, tag="rbc",
                                   name=f"rbc{half}{u}")
                    nc.vector.reciprocal_approx_fast(out=rbc[:],
                                                     in_=rbp[0:64, :])
                cols = slice(u * QT, (u + 1) * QT)
                if half == 0:
                    nc.vector.tensor_mul(o2[pr][0:64, cols],
                                         ot[0:64, :], rbc[:])
                else:
                    o2t = sbN.tile([64, QT], bf16, tag="o2t",
                                   name=f"o2t{half}{u}")
                    nc.vector.tensor_mul(o2t[:], ot[0:64, :], rbc[:])
                    nc.sync.dma_start(o2[pr][64:128, cols], o2t[:])

            def av_pieces(st_q0, st_qn, lhs, et, ecol, erow, rows):
                # accumulate AV pieces into the per-u psum banks (both halves
                # share the strip's E tile; `erow`/`ecol` locate each half's
                # scores inside it)
                for half in range(2):
                    qlo = st_q0 * BLK
                    qhi = (st_q0 + st_qn) * BLK
                    q = qlo
                    while q < qhi:
                        u = q // QT
                        qe = min(qhi, (u + 1) * QT)
                        ot = get_ot(half, u)
                        nc.tensor.matmul(
                            ot[0:65, q - u * QT:qe - u * QT],
                            lhsT=lhs[half],
                            rhs=et[erow[half]:erow[half] + rows,
                                   ecol[half] + q - qlo:ecol[half] + qe - qlo],
                            start=(done[(half, u)] == 0),
                            stop=(done[(half, u)] == npieces[u] - 1))
                        done[(half, u)] += 1
                        if done[(half, u)] == npieces[u]:
                            finish(half, u)
                        q = qe

            strips_by_u = [[] for _ in range(4)]
            for si, st in enumerate(strips):
                strips_by_u[st["q0"] * BLK // QT].append((si, st))

            for u in range(4):
                # open this q-tile's banks (emits the glob pieces)
                for half in range(2):
                    get_ot(half, u)

                # --- band strips, paired two per 2-bank scores tile ---
                bands = [s for s in strips_by_u[u] if s[1]["kind"] == "band"]
                extras = [s for s in strips_by_u[u] if s[1]["kind"] != "band"]
                for gi in range(0, len(bands), 2):
                    grp = bands[gi:gi + 2]
                    pss = psS.tile([P, 2 * QT], f32, tag="s",
                                   name=f"s{grp[0][0]}")
                    em = sbE.tile([P, 2 * QT], bf16, tag="em",
                                  name=f"em{grp[0][0]}")
                    for sl, (si, st) in enumerate(grp):
                        qlo, qn = st["q0"] * BLK, st["qn"] * BLK
                        k0 = st["k"][0] * BLK
                        for half in range(2):
                            h64 = half * 64
                            c0 = half * QT + sl * 256
                            nc.tensor.matmul(
                                pss[:, c0:c0 + qn],
                                lhsT=kt_[h64:h64 + 64, k0:k0 + 128],
                                rhs=qt_[h64:h64 + 64, qlo:qlo + qn],
                                start=True, stop=True)
                    wcols = 256 * len(grp)
                    nc.scalar.activation(em[:, 0:wcols], pss[:, 0:wcols],
                                         Exp, scale=0.125)
                    nc.scalar.activation(em[:, QT:QT + wcols],
                                         pss[:, QT:QT + wcols],
                                         Exp, scale=0.125)
                    for sl, (si, st) in enumerate(grp):
                        for half in range(2):
                            for ki in range(2):
                                for qi in range(st["qn"]):
                                    if not st["act"][ki, qi]:
                                        c0 = half * QT + sl * 256 + qi * 64
                                        nc.gpsimd.memset(
                                            em[ki * 64:(ki + 1) * 64,
                                               c0:c0 + 64], 0.0)
                        g = (st["k"][0] - 1) // 2
                        lhs = [v2b[h][:, g * 65:(g + 1) * 65]
                               for h in range(2)]
                        av_pieces(st["q0"], st["qn"], lhs, em,
                                  ecol=(sl * 256, QT + sl * 256),
                                  erow=(0, 0), rows=128)

                # --- extra strips (single k-block), A and B quadrants of
                # one bank at disjoint partition ranges ---
                for si, st in extras:
                    qlo, qn = st["q0"] * BLK, st["qn"] * BLK
                    kb = st["k"][0]
                    psx = psS.tile([P, 2 * QT], f32, tag="s", name=f"s{si}")
                    ex = sbE.tile([P, 512], bf16, tag="ex", name=f"ex{si}")
                    for half in range(2):
                        h64 = half * 64
                        nc.gpsimd.memset(
                            ex[h64:h64 + 64,
                               half * 256:half * 256 + qn], 0.0)
                        nc.tensor.matmul(
                            psx[h64:h64 + 64,
                                half * 256:half * 256 + qn],
                            lhsT=kt_[h64:h64 + 64,
                                     kb * BLK:kb * BLK + 64],
                            rhs=qt_[h64:h64 + 64, qlo:qlo + qn],
                            start=True, stop=True)
                        for qi in range(st["qn"]):
                            if st["act"][0, qi]:
                                nc.scalar.activation(
                                    ex[h64:h64 + 64,
                                       half * 256 + qi * 64:
                                       half * 256 + (qi + 1) * 64],
                                    psx[h64:h64 + 64,
                                        half * 256 + qi * 64:
                                        half * 256 + (qi + 1) * 64],
                                    Exp, scale=0.125)
                    lhs = [vxs[si][0:64, :], vxs[si][64:128, :]]
                    av_pieces(st["q0"], st["qn"], lhs, ex,
                              ecol=(0, 256), erow=(0, 64), rows=64)

        # ---------------- Phase C: partial output projection -------------
        with tc.tile_pool(name="yp", bufs=3) as yp:
            for qb in range(4):
                for mt in range(8):
                    ps = psA.tile([P, 512], f32, tag="mm512")
                    for a in range(2):
                        nc.tensor.matmul(
                            ps[:],
                            lhsT=wob[:, a * DIM + mt * P:
                                     a * DIM + (mt + 1) * P],
                            rhs=o2[a][:, qb * 512:(qb + 1) * 512],
                            start=(a == 0), stop=(a == 1))
                    yt = yp.tile([P, 512], bf16, tag="yt")
                    nc.vector.tensor_copy(yt[:], ps[:])
                    nc.sync.dma_start(
                        d_yt[mt * P:(mt + 1) * P, qb * 512:(qb + 1) * 512],
                        yt[:])


def kernel(x, Wq, bq, Wk, bk, Wv, bv, Wo, bo, src_blocks, tgt_blocks,
           _trace=False):
    global LAST_EXEC_NS, LAST_TRACE, LAST_INSTS
    x = np.asarray(x, np.float32)
    bm = _block_mask(np.asarray(src_blocks), np.asarray(tgt_blocks))
    strips = _plan_strips(bm)
    use_bias = bool(np.any(np.asarray(bq)) or np.any(np.asarray(bk))
                    or np.any(np.asarray(bv)))
    nc = _build_program(strips, use_bias)

    # host-side shard prep
    # W layout for rhs: w[p, kt*1024 + j] = W[j, kt*128 + p]
    def w_rhs(W):
        Wt = np.ascontiguousarray(np.asarray(W, np.float32).T)  # [in, out]
        return np.ascontiguousarray(
            Wt.reshape(8, P, DIM).transpose(1, 0, 2).reshape(P, 8 * DIM)
        ).astype(ml_dtypes.bfloat16)

    wq_h, wk_h, wv_h = w_rhs(Wq), w_rhs(Wk), w_rhs(Wv)
    WoT = np.asarray(Wo, np.float32).T  # [in(=64*head), out]
    x4 = x.reshape(B, NHEADS, P, DIM)

    in_maps = []
    for c in range(NCORES):
        b = c // 4
        h0 = 4 * (c % 4)
        xc = x4[b, h0:h0 + 4]                       # [4, 128, 1024]
        xt = np.ascontiguousarray(xc.transpose(0, 2, 1))  # [4, 1024, 128]
        # xt dram layout [4, 128, 8*128]: xts[i, p, kt*128+t] = x[t, kt*128+p]
        xts = np.ascontiguousarray(
            xt.reshape(HPC, 8, P, P).transpose(0, 2, 1, 3).reshape(
                HPC, P, 8 * P)).astype(ml_dtypes.bfloat16)
        wo_c = np.zeros((2, P, DIM), ml_dtypes.bfloat16)
        for a in range(2):
            r0 = 64 * (h0 + 2 * a)
            wo_c[a] = WoT[r0:r0 + 128].astype(ml_dtypes.bfloat16)
        in_maps.append({
            "xt": xts,
            "wq": wq_h, "wk": wk_h, "wv": wv_h,
            "bq": np.asarray(bq, np.float32).reshape(1, DIM),
            "bk": np.asarray(bk, np.float32).reshape(1, DIM),
            "bv": np.asarray(bv, np.float32).reshape(1, DIM),
            "wo": wo_c,
        })

    if _trace:
        try:
            import sys
            sys.path.insert(0, "/root/problem/work")
            import ntff_shim
            ntff_shim.install()
        except Exception:
            pass
    res = run_bass_kernel_spmd(nc, in_maps, core_ids=list(range(NCORES)),
                               trace=_trace)
    LAST_EXEC_NS = res.exec_time_ns
    LAST_TRACE = (res.instructions_and_trace[1]
                  if res.instructions_and_trace else None)
    LAST_INSTS = (res.instructions_and_trace[0]
                  if res.instructions_and_trace else None)

    y = np.zeros((B, S, DIM), np.float32)
    for c in range(NCORES):
        y[c // 4] += np.asarray(res.results[c]["yt"], np.float32).T
    y += np.asarray(bo, np.float32)
    return y
